# revision 1
# baseline (speedup 1.0000x reference)
"""Trainium2 Bass kernel for nn_BasicRNN_42271068127787.

3-layer LSTM (input=20, hidden=6, seq=34) + FC(204->20) + log_softmax over
batch 32768, data-parallel over 8 NeuronCores (4096 rows/core).

Layout (per core):
  - batch 4096 rows -> NB=10 chunks x BF=416 cols (4160, 64 zero-pad cols)
  - feature-major on chip: activations [feature-rows, batch-cols]
  - gates per (t, layer): one PSUM tile G2 [128, 2, 416] (2 banks):
      bank0 = [i-gates rows 0..59 | g-gates rows 64..123]
      bank1 = [f-gates | o-gates]
    from block-diagonal replicated-weight float32r matmuls (1 cyc/row).
  - walrus rules honored: DVE same-base for two SBUF inputs (outputs and
    ACT outputs may shift partitions; PSUM input exempts the rule).
  - h-state: A [64, 416] = h0 ; B [128, 416] = [h1 | h2] at rows 0/64.
    Row 60 of A/B and row 124 of B hold constant 1.0: biases are folded
    into the recurrent matmul lhsT (bias row 60), fc bias as fc_b/SEQ.
  - FC accumulated inline over t into 2 pinned PSUM tiles [100, 416]
  - log_softmax on device (no max subtraction: logits are O(1))
"""

import sys

import numpy as np

if "/opt/trn_rl_repo" not in sys.path:
    sys.path.insert(0, "/opt/trn_rl_repo")

B_TOTAL = 32768
INPUT = 20
HID = 6
SEQ = 34
CLS = 20
NCORES = 8
BC = B_TOTAL // NCORES  # 4096
NB = 10                 # batch chunks per core
BF = 410                # batch cols per chunk
BCP = NB * BF           # 4160 padded rows per core

_CACHE = {}


# ---------------------------------------------------------------- host prep

def _build_wblob(w_ih, w_hh, b_ih, b_hh, fc_w, fc_b):
    """Pack all lhsT weight tiles into one [128, WC] fp32 blob.

    Gate order in torch weights: rows 0..5=i, 6..11=f, 12..17=g, 18..23=o.
    M-layout of IG tiles: col 6c+h = i-gate, col 64+6c+h = g-gate.
    FO tiles: f / o.  Bias row: lhsT row 60 (paired with const-1.0 row 60
    of the h rhs tiles).
    """
    cols = {}
    blocks = []
    cursor = 0

    def alloc(name, n):
        nonlocal cursor
        cols[name] = cursor
        arr = np.zeros((128, n), dtype=np.float32)
        blocks.append(arr)
        cursor += n
        return arr

    def fill_gate_cols(dst, row_of, src_w, ga, gb, nin):
        for c in range(NB):
            for h in range(HID):
                for k in range(nin):
                    r = row_of(c, k)
                    dst[r, 6 * c + h] = src_w[ga * HID + h, k]
                    dst[r, 64 + 6 * c + h] = src_w[gb * HID + h, k]

    def fill_bias_row(dst, row, bsum, ga, gb):
        for c in range(NB):
            for h in range(HID):
                dst[row, 6 * c + h] = bsum[ga * HID + h]
                dst[row, 64 + 6 * c + h] = bsum[gb * HID + h]

    bsum = [b_ih[l] + b_hh[l] for l in range(3)]

    # layer 0: x feats split 0..9 / 10..19 (chunk-major rows 10c+f), h0 tile
    for half in range(2):
        for nm, ga, gb in (("x%dIG" % half, 0, 2), ("x%dFO" % half, 1, 3)):
            a = alloc(nm, 128)
            fill_gate_cols(a, lambda c, k: 10 * c + k,
                           w_ih[0][:, half * 10:half * 10 + 10], ga, gb, 10)
    for nm, ga, gb in (("hIG0", 0, 2), ("hFO0", 1, 3)):
        a = alloc(nm, 128)
        fill_gate_cols(a, lambda c, k: 6 * c + k, w_hh[0], ga, gb, HID)
        fill_bias_row(a, 60, bsum[0], ga, gb)
    # layer 1: input part (reads A = h0, bias row) and recurrent (reads B[0:64])
    for nm, src, ga, gb, brow in (("aIG1", w_ih[1], 0, 2, True),
                                  ("aFO1", w_ih[1], 1, 3, True),
                                  ("bIG1", w_hh[1], 0, 2, False),
                                  ("bFO1", w_hh[1], 1, 3, False)):
        a = alloc(nm, 128)
        fill_gate_cols(a, lambda c, k: 6 * c + k, src, ga, gb, HID)
        if brow:
            fill_bias_row(a, 60, bsum[1], ga, gb)
    # layer 2 fused: rows 0..63 = h1 block (w_ih2, bias row 60),
    #                rows 64..127 = h2 block (w_hh2)
    for nm, ga, gb in (("W2IG", 0, 2), ("W2FO", 1, 3)):
        a = alloc(nm, 128)
        for c in range(NB):
            for h in range(HID):
                for k in range(HID):
                    a[6 * c + k, 6 * c + h] = w_ih[2][ga * HID + h, k]
                    a[6 * c + k, 64 + 6 * c + h] = w_ih[2][gb * HID + h, k]
                    a[64 + 6 * c + k, 6 * c + h] = w_hh[2][ga * HID + h, k]
                    a[64 + 6 * c + k, 64 + 6 * c + h] = w_hh[2][gb * HID + h, k]
        fill_bias_row(a, 60, bsum[2], ga, gb)
    # FC: rhs is B[64:128] (base 64) -> lhsT tiles live at rows 64..127.
    # Row 124 pairs with B's const-1.0 row: fc bias / SEQ added every t.
    for t in range(SEQ):
        a = alloc("fcA%d" % t, 100)
        b = alloc("fcB%d" % t, 100)
        for c in range(NB):
            for cl in range(10):
                for h in range(HID):
                    a[64 + 6 * c + h, 10 * c + cl] = fc_w[cl, t * HID + h]
                    b[64 + 6 * c + h, 10 * c + cl] = fc_w[10 + cl, t * HID + h]
                a[124, 10 * c + cl] = fc_b[cl] / SEQ
                b[124, 10 * c + cl] = fc_b[10 + cl] / SEQ
    # block-diag ones for per-chunk logsumexp reduce/broadcast
    a = alloc("onesK", 10)      # lhsT [100, 10]: col c = 1 at rows 10c..10c+9
    b = alloc("onesM", 100)     # lhsT [10, 100]: row c = 1 at cols 10c..10c+9
    for c in range(NB):
        a[10 * c:10 * c + 10, c] = 1.0
        b[c, 10 * c:10 * c + 10] = 1.0
    # all-ones row source for the const-1.0 rows of A/B
    a = alloc("ones416", BF)
    a[:] = 1.0

    blob = np.concatenate(blocks, axis=1)
    return np.ascontiguousarray(blob), cols


def _prep_x(x_core):
    """(4096, 20, 34) -> [34, 2, 100, 416] fp32, chunk c col j <-> row c*416+j."""
    xp = np.zeros((BCP, INPUT, SEQ), dtype=np.float32)
    xp[:BC] = x_core
    xr = xp.reshape(NB, BF, INPUT, SEQ).transpose(3, 2, 0, 1)  # (34, 20, 10, 416)
    xr = xr.reshape(SEQ, 2, 10, NB, BF).transpose(0, 1, 3, 2, 4)
    return np.ascontiguousarray(xr.reshape(SEQ, 2, 100, BF))


def _unpack_out(res):
    """[2, 100, 416] -> (4096, 20)."""
    r = res.reshape(2, NB, 10, BF)          # (half, chunk, cls, col)
    r = r.transpose(1, 3, 0, 2).reshape(BCP, CLS)
    return r[:BC]


# ---------------------------------------------------------------- program

def _make_nc(wc_total, col, loop_n=1):
    import concourse.tile as tile
    from concourse import bacc, mybir

    F = mybir.dt.float32
    FR = mybir.dt.float32r
    AF = mybir.ActivationFunctionType
    Alu = mybir.AluOpType

    nc = bacc.Bacc("TRN2", target_bir_lowering=False, debug=False)
    xd = nc.declare_dram_parameter("xin", [SEQ, 2, 100, BF], FR, isOutput=False)
    wd = nc.declare_dram_parameter("win", [128, wc_total], FR, isOutput=False)
    od = nc.declare_dram_parameter("oout", [2, 100, BF], F, isOutput=True)

    with tile.TileContext(nc) as tc:
        with (
            tc.tile_pool(name="w", bufs=1) as wp,
            tc.tile_pool(name="x", bufs=4) as xp,
            tc.tile_pool(name="s", bufs=3) as sp,
            tc.tile_pool(name="st", bufs=1) as st,
            tc.tile_pool(name="g", bufs=3, space="PSUM") as gp,
            tc.tile_pool(name="fc", bufs=1, space="PSUM") as fp,
        ):
            wsb = wp.tile([128, wc_total], FR)
            nc.sync.dma_start(out=wsb[:], in_=wd[:])

            def wap(name, r0, r1, c0, c1):
                c = col[name]
                return wsb[r0:r1, c + c0:c + c1]

            import contextlib
            loop_cm = (tc.For_i(0, loop_n, 1,
                                hint_engines=(mybir.EngineType.PE,
                                              mybir.EngineType.Activation,
                                              mybir.EngineType.DVE,
                                              mybir.EngineType.SP))
                       if loop_n > 1 else contextlib.nullcontext())
            with loop_cm:
                A = st.tile([64, BF], FR, tag="A")
                Bt = st.tile([128, BF], FR, tag="B")
                # X2[l]: bank0 = tanh(g) scratch, bank1 = c state
                X2 = [st.tile([64, 2, BF], F, tag="X2%d" % l, name="X2%d" % l)
                      for l in range(3)]
                nc.vector.memset(A[:].bitcast(F), 0.0)
                nc.vector.memset(Bt[:].bitcast(F), 0.0)
                for l in range(3):
                    nc.vector.memset(X2[l][:], 0.0)
                # const-1.0 rows (bias rows) via tiny SBUF->SBUF DMAs
                nc.sync.dma_start(out=A[60:61, :], in_=wap("ones416", 60, 61, 0, BF))
                nc.sync.dma_start(out=Bt[60:61, :], in_=wap("ones416", 60, 61, 0, BF))
                nc.sync.dma_start(out=Bt[124:125, :], in_=wap("ones416", 124, 125, 0, BF))
                pa = fp.tile([100, BF], F, tag="pa")
                pb = fp.tile([100, BF], F, tag="pb")

                hdst = {0: A[0:60], 1: Bt[0:60], 2: Bt[64:124]}
                # wavefront: stage s runs layer l at t = s - l (independent
                # chains); all matmuls first (they read last stage's h), then
                # the elementwise chains, then FC on the just-written h2.
                for s_ in range(SEQ + 2):
                    if s_ < SEQ:
                        xa = xp.tile([100, BF], FR, tag="xa")
                        xb = xp.tile([100, BF], FR, tag="xb")
                        nc.sync.dma_start(out=xa[:], in_=xd[s_, 0])
                        nc.sync.dma_start(out=xb[:], in_=xd[s_, 1])
                    rhs_sets = {
                        0: [(xa[:], "x0IG", "x0FO", 100),
                            (xb[:], "x1IG", "x1FO", 100),
                            (A[:], "hIG0", "hFO0", 64)],
                        1: [(A[:], "aIG1", "aFO1", 64),
                            (Bt[0:64], "bIG1", "bFO1", 64)],
                        2: [(Bt[:], "W2IG", "W2FO", 128)],
                    }
                    live = [l for l in range(3) if 0 <= s_ - l < SEQ]
                    g2s = {}
                    for l in live:
                        g2 = gp.tile([128, 2, 512], F, tag="g2",
                                     name="g2_%d_%d" % (s_, l))
                        g2s[l] = g2
                        items = rhs_sets[l]
                        n = len(items)
                        for gi in range(2):
                            for i, (rhs, wig, wfo, K) in enumerate(items):
                                nc.tensor.matmul(g2[:, gi, 0:BF],
                                                 wap(wig if gi == 0 else wfo,
                                                     0, K, 0, 128),
                                                 rhs,
                                                 start=(i == 0),
                                                 stop=(i == n - 1))
                    for l in live:
                        g2 = g2s[l]
                        sif = sp.tile([64, 2, BF], F, tag="sif")
                        so = sp.tile([64, BF], F, tag="so")
                        z = sp.tile([64, 2, BF], F, tag="z")
                        tcl = sp.tile([64, BF], F, tag="tcl")
                        # sigmoid(i | f) in one shot (banks 0,1 of rows 0..63)
                        nc.scalar.activation(out=sif[:], in_=g2[0:64, :, 0:BF],
                                             func=AF.Sigmoid)
                        # tanh(g): rows 64..127 bank0 -> shifted to X2 bank0
                        nc.scalar.activation(out=X2[l][:, 0, :],
                                             in_=g2[64:128, 0, 0:BF],
                                             func=AF.Tanh)
                        # sigmoid(o): rows 64..127 bank1 -> shifted to 0
                        nc.scalar.activation(out=so[:], in_=g2[64:128, 1, 0:BF],
                                             func=AF.Sigmoid)
                        # z = [i*tanh_g | f*c] in one 2-bank op
                        nc.vector.tensor_mul(out=z[:], in0=sif[:], in1=X2[l][:])
                        nc.vector.tensor_add(out=X2[l][:, 1, :],
                                             in0=z[:, 0, :], in1=z[:, 1, :])
                        nc.scalar.activation(out=tcl[:], in_=X2[l][:, 1, :],
                                             func=AF.Tanh)
                        nc.vector.tensor_mul(out=hdst[l], in0=so[0:60],
                                             in1=tcl[0:60])
                    t2_ = s_ - 2
                    if 0 <= t2_ < SEQ:
                        nc.tensor.matmul(pa[:], wap("fcA%d" % t2_, 64, 128, 0, 100),
                                         Bt[64:128],
                                         start=(t2_ == 0), stop=(t2_ == SEQ - 1))
                        nc.tensor.matmul(pb[:], wap("fcB%d" % t2_, 64, 128, 0, 100),
                                         Bt[64:128],
                                         start=(t2_ == 0), stop=(t2_ == SEQ - 1))

                # ---- log_softmax tail (logits are O(1); skip max subtraction)
                ea = sp.tile([100, BF], FR, tag="sif")
                eb = sp.tile([100, BF], FR, tag="tg")
                la = sp.tile([100, BF], F, tag="la")
                lb = sp.tile([100, BF], F, tag="lb")
                nc.scalar.activation(out=la[:], in_=pa[:], func=AF.Identity)
                nc.scalar.activation(out=lb[:], in_=pb[:], func=AF.Identity)
                nc.scalar.activation(out=ea[:], in_=pa[:], func=AF.Exp)
                nc.scalar.activation(out=eb[:], in_=pb[:], func=AF.Exp)
                s = gp.tile([10, BF], F, tag="g2")
                nc.tensor.matmul(s[:], wap("onesK", 0, 100, 0, 10), ea[:],
                                 start=True, stop=False)
                nc.tensor.matmul(s[:], wap("onesK", 0, 100, 0, 10), eb[:],
                                 start=False, stop=True)
                lnz = sp.tile([10, BF], FR, tag="lnz")
                nc.scalar.activation(out=lnz[:], in_=s[:], func=AF.Ln)
                bc = gp.tile([100, BF], F, tag="g2")
                nc.tensor.matmul(bc[:], wap("onesM", 0, 10, 0, 100), lnz[:],
                                 start=True, stop=True)
                oa = sp.tile([100, BF], F, tag="la")
                ob = sp.tile([100, BF], F, tag="lb")
                nc.vector.scalar_tensor_tensor(out=oa[:], in0=bc[:], scalar=-1.0,
                                               in1=la[:], op0=Alu.mult, op1=Alu.add)
                nc.vector.scalar_tensor_tensor(out=ob[:], in0=bc[:], scalar=-1.0,
                                               in1=lb[:], op0=Alu.mult, op1=Alu.add)
                nc.sync.dma_start(out=od[0], in_=oa[:])
                nc.sync.dma_start(out=od[1], in_=ob[:])
    nc.compile()
    return nc


def _get_program(inputs, loop_n=1):
    w_ih = [inputs["w_ih%d" % l] for l in range(3)]
    w_hh = [inputs["w_hh%d" % l] for l in range(3)]
    b_ih = [inputs["b_ih%d" % l] for l in range(3)]
    b_hh = [inputs["b_hh%d" % l] for l in range(3)]
    blob, col = _build_wblob(w_ih, w_hh, b_ih, b_hh,
                             inputs["fc_w"], inputs["fc_b"])
    key = "nc%d" % loop_n
    if key not in _CACHE:
        _CACHE[key] = _make_nc(blob.shape[1], col, loop_n)
    return _CACHE[key], blob


def kernel(**inputs):
    from concourse.bass_utils import run_bass_kernel_spmd

    nc, blob = _get_program(inputs)
    x = np.asarray(inputs["x"], dtype=np.float32)
    in_maps = []
    for c in range(NCORES):
        xc = x[c * BC:(c + 1) * BC, 0]  # (4096, 20, 34)
        in_maps.append({"xin": _prep_x(xc), "win": blob})
    res = run_bass_kernel_spmd(nc, in_maps, list(range(NCORES)),
                               trace=_CACHE.get("trace", False))
    _CACHE["last_res"] = res
    out = np.empty((B_TOTAL, CLS), dtype=np.float32)
    for c in range(NCORES):
        out[c * BC:(c + 1) * BC] = _unpack_out(res.results[c]["oout"])
    return out



# revision 21
# speedup vs baseline: 1.4216x; 1.4216x over previous
"""Trainium2 Bass kernel for nn_BasicRNN_42271068127787.

3-layer LSTM (input=20, hidden=6, seq=34) + FC(204->20) + log_softmax over
batch 32768, data-parallel over 8 NeuronCores (4096 rows/core).

Layout (per core), redesigned for ACT-engine throughput (the bottleneck):
  - batch 4096 -> 21 chunks x 196 cols (4116, 20 zero-pad elems)
  - gate tensors live in per-gate PSUM regions [126, 196] (126 = 21*6
    partitions), packed on a (bank, col-offset) grid so ONE activation op
    covers a gate across all 3 live layers:
      G[128, 6, 512]: bank l      = [i_l | f_l]  (cols 0:196 | 196:392)
                      bank 3+l    = [o_l | g_l]
    ACT per stage: sigmoid(i,f) x3 layers (1 op), tanh(g) (1), sigmoid(o)
    (1), tanh(c) (1) -- free sizes 1176/588/588/588 at 126 partitions vs
    the old 64-partition layout's 6240.
  - all matmul operands fp16 (1 cyc/col at any N; fp32r needs N>=256),
    weights+x+h/c states fp16; PSUM accumulation fp32.
  - single-gate matmuls: x (L0) in 4 row-passes/gate; h-inputs single-pass
    [127 or 126 rows]; bias folded via const-1.0 row 126 of the h tile.
  - DVE chain fused across layers: z = S_if*[tg|c] (1), c' = z_i+z_f (1),
    h = S_o*tanh(c') (1); fp16 2x mode.
  - FC accumulated inline over t into 2 pinned PSUM banks, 4 chunk-groups
    (6,6,6,3); fc bias as fc_b/SEQ via const row.
  - log_softmax tail on device (no max subtraction; logits are O(1)).
"""

import sys

import numpy as np

if "/opt/trn_rl_repo" not in sys.path:
    sys.path.insert(0, "/opt/trn_rl_repo")

B_TOTAL = 32768
INPUT = 20
HID = 6
SEQ = 34
CLS = 20
NCORES = 8
BC = B_TOTAL // NCORES   # 4096
NB = 21                  # batch chunks per core
BF = 196                 # batch cols per chunk
BCP = NB * BF            # 4116 padded batch per core
XCH = (6, 6, 6, 3)       # chunks per x-matmul pass
FCG = (6, 6, 6, 3)       # chunks per FC output group
# gate -> (bank, col offset) in the G PSUM tile; torch gate index
GATES = (("i", 0), ("f", 1), ("o", 3), ("g", 2))


def _gate_region(gname, l=0):
    # -> (layer-bank, half-bank) inside Gif (i,f) or Gog (o,g) PSUM tiles
    return l, (0 if gname in ("i", "o") else 1)


_CACHE = {}


# ---------------------------------------------------------------- host prep

def _build_wblob(w_ih, w_hh, b_ih, b_hh, fc_w, fc_b):
    """Pack all lhsT weight tiles into one [128, WC] fp16 blob."""
    cols = {}
    blocks = []
    cursor = 0

    def alloc(name, n):
        nonlocal cursor
        cols[name] = cursor
        arr = np.zeros((128, n), dtype=np.float32)
        blocks.append(arr)
        cursor += n
        return arr

    bsum = [b_ih[l] + b_hh[l] for l in range(3)]

    # L0 x tiles: pass p covers chunks 6p..6p+ncp-1; row cc*20+k -> out col
    # 36p+cc*6+h with w_ih0[gt*6+h, k].  Out cols span the full 126-row
    # region (PE requires out base partition 0/32/64), zero elsewhere.
    for gname, gt in GATES:
        for p in range(4):
            ncp = XCH[p]
            a = alloc("x%s%d" % (gname, p), 126)
            for cc in range(ncp):
                o = 36 * p + cc * 6
                a[cc * 20:cc * 20 + 20, o:o + 6] = \
                    w_ih[0][gt * 6:gt * 6 + 6, :].T
    # h-input lhsT tiles [127 or 126, 126], block-diag per chunk; bias on
    # row 126 for the tiles that pair with the const-1.0 rhs row.
    def hblk(name, w, gt, bias):
        a = alloc(name, 126)
        for c in range(NB):
            a[6 * c:6 * c + 6, 6 * c:6 * c + 6] = w[gt * 6:gt * 6 + 6, :].T
        if bias is not None:
            for c in range(NB):
                a[126, 6 * c:6 * c + 6] = bias[gt * 6:gt * 6 + 6]

    for gname, gt in GATES:
        hblk("h0%s" % gname, w_hh[0], gt, bsum[0])
        hblk("a1%s" % gname, w_ih[1], gt, bsum[1])
        hblk("b1%s" % gname, w_hh[1], gt, None)
        hblk("a2%s" % gname, w_ih[2], gt, bsum[2])
        hblk("b2%s" % gname, w_hh[2], gt, None)
    # FC tiles per (t, group): rows 6c+h -> col cc*20+cl
    for t in range(SEQ):
        for j in range(4):
            ncj = FCG[j]
            a = alloc("fc%d_%d" % (t, j), 20 * ncj)
            for cc in range(ncj):
                c = 6 * j + cc
                a[6 * c:6 * c + 6, cc * 20:cc * 20 + 20] = \
                    fc_w[:, t * 6:t * 6 + 6].T
                a[126, cc * 20:cc * 20 + 20] = fc_b / SEQ
    a = alloc("ones", BF)
    a[:] = 1.0
    # tail reduce/broadcast ones (out cols span full region, zero elsewhere)
    for j in range(4):
        ncj = FCG[j]
        a = alloc("redK%d" % j, NB)
        for cc in range(ncj):
            a[cc * 20:cc * 20 + 20, 6 * j + cc] = 1.0
        a = alloc("redM%d" % j, 20 * ncj)
        for cc in range(ncj):
            a[6 * j + cc, cc * 20:cc * 20 + 20] = 1.0

    blob = np.concatenate(blocks, axis=1).astype(np.float16)
    return np.ascontiguousarray(blob), cols


def _prep_x(x_core):
    """(4096, 20, 34) -> [34, 120, 4, 196] fp16; pass p rows cc*20+f."""
    xp = np.zeros((BCP, INPUT, SEQ), dtype=np.float32)
    xp[:BC] = x_core
    arr = xp.reshape(NB, BF, INPUT, SEQ).transpose(3, 0, 2, 1)  # (34,21,20,196)
    a24 = np.zeros((SEQ, 24, INPUT, BF), dtype=np.float32)
    a24[:, :NB] = arr
    a24 = a24.reshape(SEQ, 4, 6 * INPUT, BF).transpose(0, 2, 1, 3)
    return np.ascontiguousarray(a24.astype(np.float16))  # (34, 120, 4, 196)


def _unpack_out(od):
    """[120, 4, 196] f32 -> (4096, 20)."""
    r = od.reshape(6, CLS, 4, BF).transpose(2, 0, 3, 1)  # (grp, cc, col, cls)
    return r.reshape(24 * BF, CLS)[:BC]


# ---------------------------------------------------------------- program

def _make_nc(wc_total, col):
    import concourse.tile as tile
    from concourse import bacc, mybir

    F = mybir.dt.float32
    H16 = mybir.dt.float16
    AF = mybir.ActivationFunctionType
    Alu = mybir.AluOpType

    nc = bacc.Bacc("TRN2", target_bir_lowering=False, debug=False)
    xd = nc.declare_dram_parameter("xin", [SEQ, 120, 4, BF], H16, isOutput=False)
    wd = nc.declare_dram_parameter("win", [128, wc_total], H16, isOutput=False)
    od = nc.declare_dram_parameter("oout", [120, 4, BF], F, isOutput=True)

    with tile.TileContext(nc) as tc:
        with (
            tc.tile_pool(name="w", bufs=1) as wp,
            tc.tile_pool(name="x", bufs=4) as xp,
            tc.tile_pool(name="s", bufs=2) as sp,
            tc.tile_pool(name="st", bufs=1) as st,
            tc.tile_pool(name="g", bufs=1, space="PSUM") as gp,
            tc.tile_pool(name="fc", bufs=1, space="PSUM") as fp,
        ):
            wsb = wp.tile([128, wc_total], H16)
            # chunked weight DMA so early stages start before FC tiles land
            w_splits = [0, col["fc0_0"], col["fc6_0"], col["fc17_0"], wc_total]
            for a, b in zip(w_splits[:-1], w_splits[1:]):
                nc.sync.dma_start(out=wsb[:, a:b], in_=wd[:, a:b])

            def wap(name, r0, r1, c0, c1):
                c = col[name]
                return wsb[r0:r1, c + c0:c + c1]

            # persistent state; dim layout [part, layer, slot, col].
            # Ht double-buffered by stage parity so FC(s) can be emitted a
            # stage late (off the critical path) while still reading h2(s).
            Hs = [st.tile([128, 3, BF], H16, tag="H%d" % k, name="H%d" % k)
                  for k in range(2)]
            Tt = st.tile([128, 3, 2, BF], H16, tag="T")  # slot 0=tanh(g), 1=c
            Sif = st.tile([128, 3, 2, BF], H16, tag="Sif")  # slot 0=sig_i, 1=sig_f
            So = st.tile([128, 3, BF], H16, tag="So")
            TC = st.tile([128, 3, BF], H16, tag="TC")
            Zt = st.tile([128, 3, 2, BF], H16, tag="Z")
            for k in range(2):
                nc.vector.memset(Hs[k][:], 0.0)
                for l in range(3):
                    # bias row 126 = 1.0 (engine ops can't address partition
                    # 126 directly; SBUF->SBUF DMA can)
                    nc.sync.dma_start(out=Hs[k][126:127, l, :],
                                      in_=wap("ones", 126, 127, 0, BF))
            nc.vector.memset(Tt[:], 0.0)

            # separate PSUM tiles so WAR deps (tile-granular) don't serialize
            # o/g matmuls behind sigmoid(i,f) reads
            Gif = gp.tile([128, 3, 2, 256], F, tag="Gif", name="Gif")
            Gog = gp.tile([128, 3, 2, 256], F, tag="Gog", name="Gog")
            FCp = fp.tile([128, 2, 2, 256], F, tag="FC")

            def fc_region(j, r0, r1):
                return FCp[r0:r1, j // 2, j % 2, 0:BF]

            def mm(out, lhsT, rhs, start, stop):
                nc.tensor.matmul(out, lhsT, rhs, start=start, stop=stop,
                                 skip_group_check=True)

            def emit_x(s_, xa):
                # x matmuls (L0): 4 chunk-passes per gate accumulating [0:126]
                for gname, _ in GATES:
                    _, hf = _gate_region(gname, 0)
                    gtile = Gif if gname in ("i", "f") else Gog
                    for p in range(4):
                        ncp = XCH[p]
                        mm(gtile[0:126, 0, hf, 0:BF],
                           wap("x%s%d" % (gname, p), 0, 20 * ncp, 0, 126),
                           xa[0:20 * ncp, p, :], start=(p == 0), stop=False)

            xtiles = {}
            for s_ in range(SEQ + 2):
                l0, l1 = max(0, s_ - (SEQ - 1)), min(2, s_)
                Hp = Hs[s_ % 2]       # h(s-1) inputs
                Hn = Hs[(s_ + 1) % 2]  # h(s) outputs
                if s_ == 0:
                    for t in (0, 1):
                        xa = xp.tile([120, 4, BF], H16, tag="xa",
                                     name="xa%d" % t)
                        nc.sync.dma_start(out=xa[:], in_=xd[t])
                        xtiles[t] = xa
                    emit_x(0, xtiles[0])
                # h-sourced matmuls: i/f gates first (they gate the ACT chain)
                for gset in (("i", "f"), ("o", "g")):
                    gtile = Gif if gset[0] == "i" else Gog
                    for gname in gset:
                        for l in range(l0, l1 + 1):
                            _, hf = _gate_region(gname, l)
                            out = gtile[0:126, l, hf, 0:BF]
                            x_open = (l == 0 and s_ < SEQ)
                            if l == 0:
                                mm(out, wap("h0%s" % gname, 0, 127, 0, 126),
                                   Hp[0:127, 0, :], start=not x_open, stop=True)
                            else:
                                nm = ("a1", "b1") if l == 1 else ("a2", "b2")
                                mm(out, wap("%s%s" % (nm[0], gname), 0, 127, 0, 126),
                                   Hp[0:127, l - 1, :], start=True, stop=False)
                                mm(out, wap("%s%s" % (nm[1], gname), 0, 126, 0, 126),
                                   Hp[0:126, l, :], start=False, stop=True)
                    if gset[0] == "i":
                        # sigmoid(i|f) across live layers: one fused op
                        nc.scalar.activation(
                            out=Sif[0:126, l0:l1 + 1, :, :],
                            in_=Gif[0:126, l0:l1 + 1, 0:2, 0:BF], func=AF.Sigmoid)
                # tanh(g) -> T slot 0; sigmoid(o) -> So
                nc.scalar.activation(out=Tt[0:126, l0:l1 + 1, 0:1, :],
                                     in_=Gog[0:126, l0:l1 + 1, 1:2, 0:BF],
                                     func=AF.Tanh)
                nc.scalar.activation(out=So[0:126, l0:l1 + 1, :],
                                     in_=Gog[0:126, l0:l1 + 1, 0, 0:BF],
                                     func=AF.Sigmoid)
                # z_f = sig_f*c right after sigmoid (no tanh_g dep), then
                # z_i = sig_i*tanh_g; c' = z_i + z_f; h = sig_o*tanh(c')
                nc.vector.tensor_mul(out=Zt[0:126, l0:l1 + 1, 1:2, :],
                                     in0=Sif[0:126, l0:l1 + 1, 1:2, :],
                                     in1=Tt[0:126, l0:l1 + 1, 1:2, :])
                nc.vector.tensor_mul(out=Zt[0:126, l0:l1 + 1, 0:1, :],
                                     in0=Sif[0:126, l0:l1 + 1, 0:1, :],
                                     in1=Tt[0:126, l0:l1 + 1, 0:1, :])
                nc.vector.tensor_add(out=Tt[0:126, l0:l1 + 1, 1:2, :],
                                     in0=Zt[0:126, l0:l1 + 1, 0:1, :],
                                     in1=Zt[0:126, l0:l1 + 1, 1:2, :])
                nc.scalar.activation(out=TC[0:126, l0:l1 + 1, :],
                                     in_=Tt[0:126, l0:l1 + 1, 1, :],
                                     func=AF.Tanh)
                nc.vector.tensor_mul(out=Hn[0:126, l0:l1 + 1, :],
                                     in0=So[0:126, l0:l1 + 1, :],
                                     in1=TC[0:126, l0:l1 + 1, :])
                # off-chain PE work, emitted after the chain ops: x(s+1),
                # prefetch DMA(s+2), FC(s-1) on Hs[(s-1+1)%2] (not yet
                # overwritten thanks to double buffering)
                if s_ + 1 < SEQ:
                    emit_x(s_ + 1, xtiles[s_ + 1])
                if s_ + 2 < SEQ:
                    xa = xp.tile([120, 4, BF], H16, tag="xa",
                                 name="xa%d" % (s_ + 2))
                    nc.sync.dma_start(out=xa[:], in_=xd[s_ + 2])
                    xtiles[s_ + 2] = xa
                t2 = s_ - 3
                if 0 <= t2 < SEQ:
                    for j in range(4):
                        ncj = FCG[j]
                        mm(fc_region(j, 0, 20 * ncj),
                           wap("fc%d_%d" % (t2, j), 0, 127, 0, 20 * ncj),
                           Hs[s_ % 2][0:127, 2, :],
                           start=(t2 == 0), stop=(t2 == SEQ - 1))
            # flush the last FC step (t2 = SEQ-1, h2 written at stage SEQ+1)
            t2 = SEQ - 1
            for j in range(4):
                ncj = FCG[j]
                mm(fc_region(j, 0, 20 * ncj),
                   wap("fc%d_%d" % (t2, j), 0, 127, 0, 20 * ncj),
                   Hs[(SEQ + 2) % 2][0:127, 2, :],
                   start=(t2 == 0), stop=(t2 == SEQ - 1))

            # ---- log_softmax tail (logits O(1); skip max subtraction)
            Lsb = sp.tile([128, 4, BF], F, tag="Lsb")
            Esb = sp.tile([128, 4, BF], H16, tag="Esb")
            for j in range(4):
                rj = 20 * FCG[j]
                nc.scalar.activation(out=Lsb[0:rj, j, :], in_=fc_region(j, 0, rj),
                                     func=AF.Identity)
                nc.scalar.activation(out=Esb[0:rj, j, :], in_=fc_region(j, 0, rj),
                                     func=AF.Exp)
            s_ps = gp.tile([21, BF], F, tag="Gif", name="Gsum")
            for j in range(4):
                ncj = FCG[j]
                mm(s_ps[0:21, :], wap("redK%d" % j, 0, 20 * ncj, 0, NB),
                   Esb[0:20 * ncj, j, :], start=(j == 0), stop=(j == 3))
            lnz = sp.tile([21, BF], H16, tag="lnz")
            nc.scalar.activation(out=lnz[:], in_=s_ps[0:21, :], func=AF.Ln)
            bc = gp.tile([128, 4, 256], F, tag="Gog", name="Gbc")
            for j in range(4):
                ncj = FCG[j]
                mm(bc[0:20 * ncj, j, 0:BF], wap("redM%d" % j, 0, NB, 0, 20 * ncj),
                   lnz[0:21, :], start=True, stop=True)
            Osb = sp.tile([128, 4, BF], F, tag="Osb")
            for j in range(4):
                rj = 20 * FCG[j]
                nc.vector.scalar_tensor_tensor(
                    out=Osb[0:rj, j, :], in0=bc[0:rj, j, 0:BF], scalar=-1.0,
                    in1=Lsb[0:rj, j, :], op0=Alu.mult, op1=Alu.add)
            nc.sync.dma_start(out=od[:], in_=Osb[0:120, :, :])
    nc.compile()
    return nc


def _get_program(inputs):
    w_ih = [inputs["w_ih%d" % l] for l in range(3)]
    w_hh = [inputs["w_hh%d" % l] for l in range(3)]
    b_ih = [inputs["b_ih%d" % l] for l in range(3)]
    b_hh = [inputs["b_hh%d" % l] for l in range(3)]
    blob, col = _build_wblob(w_ih, w_hh, b_ih, b_hh,
                             inputs["fc_w"], inputs["fc_b"])
    if "nc1" not in _CACHE:
        _CACHE["nc1"] = _make_nc(blob.shape[1], col)
    return _CACHE["nc1"], blob


def kernel(**inputs):
    from concourse.bass_utils import run_bass_kernel_spmd

    nc, blob = _get_program(inputs)
    x = np.asarray(inputs["x"], dtype=np.float32)
    in_maps = []
    for c in range(NCORES):
        xc = x[c * BC:(c + 1) * BC, 0]  # (4096, 20, 34)
        in_maps.append({"xin": _prep_x(xc), "win": blob})
    res = run_bass_kernel_spmd(nc, in_maps, list(range(NCORES)),
                               trace=_CACHE.get("trace", False))
    _CACHE["last_res"] = res
    out = np.empty((B_TOTAL, CLS), dtype=np.float32)
    for c in range(NCORES):
        out[c * BC:(c + 1) * BC] = _unpack_out(res.results[c]["oout"])
    return out


# revision 27
# speedup vs baseline: 1.4847x; 1.0444x over previous
"""Trainium2 Bass kernel for nn_BasicRNN_42271068127787.

3-layer LSTM (input=20, hidden=6, seq=34) + FC(204->20) + log_softmax over
batch 32768, data-parallel over 8 NeuronCores (4096 rows/core).

Layout (per core), redesigned for ACT-engine throughput (the bottleneck):
  - batch 4096 -> 21 chunks x 196 cols (4116, 20 zero-pad elems)
  - gate tensors live in per-gate PSUM regions [126, 196] (126 = 21*6
    partitions), packed on a (bank, col-offset) grid so ONE activation op
    covers a gate across all 3 live layers:
      G[128, 6, 512]: bank l      = [i_l | f_l]  (cols 0:196 | 196:392)
                      bank 3+l    = [o_l | g_l]
    ACT per stage: sigmoid(i,f) x3 layers (1 op), tanh(g) (1), sigmoid(o)
    (1), tanh(c) (1) -- free sizes 1176/588/588/588 at 126 partitions vs
    the old 64-partition layout's 6240.
  - all matmul operands fp16 (1 cyc/col at any N; fp32r needs N>=256),
    weights+x+h/c states fp16; PSUM accumulation fp32.
  - single-gate matmuls: x (L0) in 4 row-passes/gate; h-inputs single-pass
    [127 or 126 rows]; bias folded via const-1.0 row 126 of the h tile.
  - DVE chain fused across layers: z = S_if*[tg|c] (1), c' = z_i+z_f (1),
    h = S_o*tanh(c') (1); fp16 2x mode.
  - FC accumulated inline over t into 2 pinned PSUM banks, 4 chunk-groups
    (6,6,6,3); fc bias as fc_b/SEQ via const row.
  - log_softmax tail on device (no max subtraction; logits are O(1)).
"""

import sys

import numpy as np

if "/opt/trn_rl_repo" not in sys.path:
    sys.path.insert(0, "/opt/trn_rl_repo")

B_TOTAL = 32768
INPUT = 20
HID = 6
SEQ = 34
CLS = 20
NCORES = 8
BC = B_TOTAL // NCORES   # 4096
NB = 21                  # batch chunks per core
BF = 196                 # batch cols per chunk
BCP = NB * BF            # 4116 padded batch per core
XCH = (6, 6, 6, 3)       # chunks per x-matmul pass
FCG = (6, 6, 6, 3)       # chunks per FC output group
# gate -> (bank, col offset) in the G PSUM tile; torch gate index
GATES = (("i", 0), ("f", 1), ("o", 3), ("g", 2))


def _gate_region(gname, l=0):
    # -> (layer-bank, half-bank) inside Gif (i,f) or Gog (o,g) PSUM tiles
    return l, (0 if gname in ("i", "o") else 1)


_CACHE = {}


# ---------------------------------------------------------------- host prep

def _build_wblob(w_ih, w_hh, b_ih, b_hh, fc_w, fc_b):
    """Pack all lhsT weight tiles into one [128, WC] fp16 blob."""
    cols = {}
    blocks = []
    cursor = 0

    def alloc(name, n):
        nonlocal cursor
        cols[name] = cursor
        arr = np.zeros((128, n), dtype=np.float32)
        blocks.append(arr)
        cursor += n
        return arr

    bsum = [b_ih[l] + b_hh[l] for l in range(3)]

    # DMA-priority order: ones + h-tiles first (stage 0 needs them), then
    # x tiles, then FC per t, then tail ones.
    a = alloc("ones", BF)
    a[:] = 1.0

    # h-input lhsT tiles [127 or 126, 126], block-diag per chunk; bias on
    # row 126 for the tiles that pair with the const-1.0 rhs row.
    def hblk(name, w, gt, bias):
        a = alloc(name, 126)
        for c in range(NB):
            a[6 * c:6 * c + 6, 6 * c:6 * c + 6] = w[gt * 6:gt * 6 + 6, :].T
        if bias is not None:
            for c in range(NB):
                a[126, 6 * c:6 * c + 6] = bias[gt * 6:gt * 6 + 6]

    for gname, gt in GATES:
        hblk("h0%s" % gname, w_hh[0], gt, bsum[0])
        hblk("a1%s" % gname, w_ih[1], gt, bsum[1])
        hblk("b1%s" % gname, w_hh[1], gt, None)
        hblk("a2%s" % gname, w_ih[2], gt, bsum[2])
        hblk("b2%s" % gname, w_hh[2], gt, None)
    # L0 x tiles: pass p covers chunks 6p..6p+ncp-1; row cc*20+k -> out col
    # 36p+cc*6+h with w_ih0[gt*6+h, k].  Out cols span the full 126-row
    # region (PE requires out base partition 0/32/64), zero elsewhere.
    for gname, gt in GATES:
        for p in range(4):
            ncp = XCH[p]
            a = alloc("x%s%d" % (gname, p), 126)
            for cc in range(ncp):
                o = 36 * p + cc * 6
                a[cc * 20:cc * 20 + 20, o:o + 6] = \
                    w_ih[0][gt * 6:gt * 6 + 6, :].T
    # FC tiles per (t, group): rows 6c+h -> col cc*20+cl
    for t in range(SEQ):
        for j in range(4):
            ncj = FCG[j]
            a = alloc("fc%d_%d" % (t, j), 20 * ncj)
            for cc in range(ncj):
                c = 6 * j + cc
                a[6 * c:6 * c + 6, cc * 20:cc * 20 + 20] = \
                    fc_w[:, t * 6:t * 6 + 6].T
                a[126, cc * 20:cc * 20 + 20] = fc_b / SEQ
    # tail reduce/broadcast ones (out cols span full region, zero elsewhere)
    for j in range(4):
        ncj = FCG[j]
        a = alloc("redK%d" % j, NB)
        for cc in range(ncj):
            a[cc * 20:cc * 20 + 20, 6 * j + cc] = 1.0
        a = alloc("redM%d" % j, 20 * ncj)
        for cc in range(ncj):
            a[6 * j + cc, cc * 20:cc * 20 + 20] = 1.0

    blob = np.concatenate(blocks, axis=1).astype(np.float16)
    return np.ascontiguousarray(blob), cols


def _prep_x(x_core):
    """(4096, 20, 34) -> [34, 120, 4, 196] fp16; pass p rows cc*20+f."""
    xp = np.zeros((BCP, INPUT, SEQ), dtype=np.float32)
    xp[:BC] = x_core
    arr = xp.reshape(NB, BF, INPUT, SEQ).transpose(3, 0, 2, 1)  # (34,21,20,196)
    a24 = np.zeros((SEQ, 24, INPUT, BF), dtype=np.float32)
    a24[:, :NB] = arr
    a24 = a24.reshape(SEQ, 4, 6 * INPUT, BF).transpose(0, 2, 1, 3)
    return np.ascontiguousarray(a24.astype(np.float16))  # (34, 120, 4, 196)


def _unpack_out(od):
    """[120, 4, 196] f32 -> (4096, 20)."""
    r = od.reshape(6, CLS, 4, BF).transpose(2, 0, 3, 1)  # (grp, cc, col, cls)
    return r.reshape(24 * BF, CLS)[:BC]


# ---------------------------------------------------------------- program

def _make_nc(wc_total, col):
    import concourse.tile as tile
    from concourse import bacc, mybir

    F = mybir.dt.float32
    H16 = mybir.dt.float16
    AF = mybir.ActivationFunctionType
    Alu = mybir.AluOpType

    nc = bacc.Bacc("TRN2", target_bir_lowering=False, debug=False)
    xd = nc.declare_dram_parameter("xin", [SEQ, 120, 4, BF], H16, isOutput=False)
    wd = nc.declare_dram_parameter("win", [128, wc_total], H16, isOutput=False)
    od = nc.declare_dram_parameter("oout", [120, 4, BF], F, isOutput=True)

    with tile.TileContext(nc) as tc:
        with (
            tc.tile_pool(name="w", bufs=1) as wp,
            tc.tile_pool(name="x", bufs=4) as xp,
            tc.tile_pool(name="s", bufs=2) as sp,
            tc.tile_pool(name="st", bufs=1) as st,
            tc.tile_pool(name="g", bufs=1, space="PSUM") as gp,
            tc.tile_pool(name="fc", bufs=1, space="PSUM") as fp,
        ):
            wsb = wp.tile([128, wc_total], H16)
            # chunked weight DMA so early stages start before FC tiles land
            w_splits = [0, col["xi0"], col["fc0_0"], col["fc6_0"],
                        col["fc17_0"], wc_total]
            for a, b in zip(w_splits[:-1], w_splits[1:]):
                nc.sync.dma_start(out=wsb[:, a:b], in_=wd[:, a:b])

            def wap(name, r0, r1, c0, c1):
                c = col[name]
                return wsb[r0:r1, c + c0:c + c1]

            # persistent state, per stream (cols 98k:98k+98 of each chunk);
            # dim layout [part, layer, slot, col].  H double-buffered by
            # stage parity so FC(s) can be emitted a stage late (off the
            # critical path) while still reading h2(s).
            BS = BF // 2  # 98 cols per stream
            Hs, Tt, Sif, So, TC, Zt = [], [], [], [], [], []
            for k in range(2):
                Hs.append([st.tile([128, 3, BS], H16, tag="H%d%d" % (k, p),
                                   name="H%d%d" % (k, p)) for p in range(2)])
                Tt.append(st.tile([128, 3, 2, BS], H16, tag="T%d" % k,
                                  name="T%d" % k))   # slot 0=tanh(g), 1=c
                Sif.append(st.tile([128, 3, 2, BS], H16, tag="S%d" % k,
                                   name="S%d" % k))  # slot 0=sig_i, 1=sig_f
                So.append(st.tile([128, 3, BS], H16, tag="O%d" % k,
                                  name="O%d" % k))
                TC.append(st.tile([128, 3, BS], H16, tag="C%d" % k,
                                  name="C%d" % k))
                Zt.append(st.tile([128, 3, 2, BS], H16, tag="Z%d" % k,
                                  name="Z%d" % k))
                for p in range(2):
                    nc.vector.memset(Hs[k][p][:], 0.0)
                    for l in range(3):
                        # bias row 126 = 1.0 (engine ops can't address
                        # partition 126; SBUF->SBUF DMA can)
                        nc.sync.dma_start(out=Hs[k][p][126:127, l, :],
                                          in_=wap("ones", 126, 127, 0, BS))
                nc.vector.memset(Tt[k][:], 0.0)

            # separate PSUM tiles so WAR deps (tile-granular) don't serialize
            # o/g matmuls behind sigmoid(i,f) reads
            Gif = gp.tile([128, 3, 2, 256], F, tag="Gif", name="Gif")
            Gog = gp.tile([128, 3, 2, 256], F, tag="Gog", name="Gog")
            FCp = fp.tile([128, 2, 2, 256], F, tag="FC")

            def fc_region(j, r0, r1, c0=0, c1=BF):
                return FCp[r0:r1, j // 2, j % 2, c0:c1]

            def mm(out, lhsT, rhs, start, stop):
                nc.tensor.matmul(out, lhsT, rhs, start=start, stop=stop,
                                 skip_group_check=True)

            def emit_x(s_, xa):
                # x matmuls (L0): 4 chunk-passes per gate accumulating [0:126]
                for gname, _ in GATES:
                    _, hf = _gate_region(gname, 0)
                    gtile = Gif if gname in ("i", "f") else Gog
                    for p in range(4):
                        ncp = XCH[p]
                        mm(gtile[0:126, 0, hf, 0:BF],
                           wap("x%s%d" % (gname, p), 0, 20 * ncp, 0, 126),
                           xa[0:20 * ncp, p, :], start=(p == 0), stop=False)

            def stream_chain(s_, k, l0, l1):
                """One stream's per-stage recurrence: h-matmuls + sigmoid/
                tanh chain on cols 98k:98k+98."""
                c0, c1 = BS * k, BS * k + BS
                Hp = Hs[k][s_ % 2]        # h(s-1) inputs
                Hn = Hs[k][(s_ + 1) % 2]  # h(s) outputs

                def hmm(gset, gtile):
                    for gname in gset:
                        for l in range(l0, l1 + 1):
                            _, hf = _gate_region(gname, l)
                            out = gtile[0:126, l, hf, c0:c1]
                            x_open = (l == 0 and s_ < SEQ)
                            if l == 0:
                                mm(out, wap("h0%s" % gname, 0, 127, 0, 126),
                                   Hp[0:127, 0, :], start=not x_open, stop=True)
                            else:
                                nm = ("a1", "b1") if l == 1 else ("a2", "b2")
                                mm(out,
                                   wap("%s%s" % (nm[0], gname), 0, 127, 0, 126),
                                   Hp[0:127, l - 1, :], start=True, stop=False)
                                mm(out,
                                   wap("%s%s" % (nm[1], gname), 0, 126, 0, 126),
                                   Hp[0:126, l, :], start=False, stop=True)

                hmm(("i", "f"), Gif)
                # sigmoid(i|f) across live layers: one fused op
                nc.scalar.activation(out=Sif[k][0:126, l0:l1 + 1, :, :],
                                     in_=Gif[0:126, l0:l1 + 1, 0:2, c0:c1],
                                     func=AF.Sigmoid)
                hmm(("o", "g"), Gog)
                # z_f = sig_f*c right after sigmoid (no tanh_g dep)
                nc.vector.tensor_mul(out=Zt[k][0:126, l0:l1 + 1, 1:2, :],
                                     in0=Sif[k][0:126, l0:l1 + 1, 1:2, :],
                                     in1=Tt[k][0:126, l0:l1 + 1, 1:2, :])
                # tanh(g) -> T slot 0; sigmoid(o) -> So
                nc.scalar.activation(out=Tt[k][0:126, l0:l1 + 1, 0:1, :],
                                     in_=Gog[0:126, l0:l1 + 1, 1:2, c0:c1],
                                     func=AF.Tanh)
                nc.scalar.activation(out=So[k][0:126, l0:l1 + 1, :],
                                     in_=Gog[0:126, l0:l1 + 1, 0, c0:c1],
                                     func=AF.Sigmoid)
                # z_i = sig_i*tanh_g; c' = z_i + z_f; h = sig_o*tanh(c')
                nc.vector.tensor_mul(out=Zt[k][0:126, l0:l1 + 1, 0:1, :],
                                     in0=Sif[k][0:126, l0:l1 + 1, 0:1, :],
                                     in1=Tt[k][0:126, l0:l1 + 1, 0:1, :])
                nc.vector.tensor_add(out=Tt[k][0:126, l0:l1 + 1, 1:2, :],
                                     in0=Zt[k][0:126, l0:l1 + 1, 0:1, :],
                                     in1=Zt[k][0:126, l0:l1 + 1, 1:2, :])
                nc.scalar.activation(out=TC[k][0:126, l0:l1 + 1, :],
                                     in_=Tt[k][0:126, l0:l1 + 1, 1, :],
                                     func=AF.Tanh)
                nc.vector.tensor_mul(out=Hn[0:126, l0:l1 + 1, :],
                                     in0=So[k][0:126, l0:l1 + 1, :],
                                     in1=TC[k][0:126, l0:l1 + 1, :])

            xtiles = {}
            for s_ in range(SEQ + 2):
                l0, l1 = max(0, s_ - (SEQ - 1)), min(2, s_)
                if s_ == 0:
                    for t in (0, 1):
                        xa = xp.tile([120, 4, BF], H16, tag="xa",
                                     name="xa%d" % t)
                        nc.sync.dma_start(out=xa[:], in_=xd[t])
                        xtiles[t] = xa
                    emit_x(0, xtiles[0])
                stream_chain(s_, 0, l0, l1)
                stream_chain(s_, 1, l0, l1)
                # off-chain PE work, emitted after the chain ops: x(s+1),
                # prefetch DMA(s+2), FC(s-1) on Hs[k][(s-1+1)%2] (not yet
                # overwritten thanks to double buffering)
                if s_ + 1 < SEQ:
                    emit_x(s_ + 1, xtiles[s_ + 1])
                if s_ + 2 < SEQ:
                    xa = xp.tile([120, 4, BF], H16, tag="xa",
                                 name="xa%d" % (s_ + 2))
                    nc.sync.dma_start(out=xa[:], in_=xd[s_ + 2])
                    xtiles[s_ + 2] = xa
                t2 = s_ - 3
                if 0 <= t2 < SEQ:
                    for k in range(2):
                        for j in range(4):
                            ncj = FCG[j]
                            mm(fc_region(j, 0, 20 * ncj, BS * k, BS * k + BS),
                               wap("fc%d_%d" % (t2, j), 0, 127, 0, 20 * ncj),
                               Hs[k][s_ % 2][0:127, 2, :],
                               start=(t2 == 0), stop=(t2 == SEQ - 1))
            # flush the last FC step (t2 = SEQ-1, h2 written at stage SEQ+1)
            t2 = SEQ - 1
            for k in range(2):
                for j in range(4):
                    ncj = FCG[j]
                    mm(fc_region(j, 0, 20 * ncj, BS * k, BS * k + BS),
                       wap("fc%d_%d" % (t2, j), 0, 127, 0, 20 * ncj),
                       Hs[k][(SEQ + 2) % 2][0:127, 2, :],
                       start=(t2 == 0), stop=(t2 == SEQ - 1))

            # ---- log_softmax tail (logits O(1); skip max subtraction)
            Lsb = sp.tile([128, 4, BF], F, tag="Lsb")
            Esb = sp.tile([128, 4, BF], H16, tag="Esb")
            for j in range(4):
                rj = 20 * FCG[j]
                nc.scalar.activation(out=Lsb[0:rj, j, :], in_=fc_region(j, 0, rj),
                                     func=AF.Identity)
                nc.scalar.activation(out=Esb[0:rj, j, :], in_=fc_region(j, 0, rj),
                                     func=AF.Exp)
            s_ps = gp.tile([21, BF], F, tag="Gif", name="Gsum")
            for j in range(4):
                ncj = FCG[j]
                mm(s_ps[0:21, :], wap("redK%d" % j, 0, 20 * ncj, 0, NB),
                   Esb[0:20 * ncj, j, :], start=(j == 0), stop=(j == 3))
            lnz = sp.tile([21, BF], H16, tag="lnz")
            nc.scalar.activation(out=lnz[:], in_=s_ps[0:21, :], func=AF.Ln)
            bc = gp.tile([128, 4, 256], F, tag="Gog", name="Gbc")
            for j in range(4):
                ncj = FCG[j]
                mm(bc[0:20 * ncj, j, 0:BF], wap("redM%d" % j, 0, NB, 0, 20 * ncj),
                   lnz[0:21, :], start=True, stop=True)
            Osb = sp.tile([128, 4, BF], F, tag="Osb")
            for j in range(4):
                rj = 20 * FCG[j]
                nc.vector.scalar_tensor_tensor(
                    out=Osb[0:rj, j, :], in0=bc[0:rj, j, 0:BF], scalar=-1.0,
                    in1=Lsb[0:rj, j, :], op0=Alu.mult, op1=Alu.add)
            nc.sync.dma_start(out=od[:], in_=Osb[0:120, :, :])
    nc.compile()
    return nc


def _get_program(inputs):
    w_ih = [inputs["w_ih%d" % l] for l in range(3)]
    w_hh = [inputs["w_hh%d" % l] for l in range(3)]
    b_ih = [inputs["b_ih%d" % l] for l in range(3)]
    b_hh = [inputs["b_hh%d" % l] for l in range(3)]
    blob, col = _build_wblob(w_ih, w_hh, b_ih, b_hh,
                             inputs["fc_w"], inputs["fc_b"])
    if "nc1" not in _CACHE:
        _CACHE["nc1"] = _make_nc(blob.shape[1], col)
    return _CACHE["nc1"], blob


def kernel(**inputs):
    from concourse.bass_utils import run_bass_kernel_spmd

    nc, blob = _get_program(inputs)
    x = np.asarray(inputs["x"], dtype=np.float32)
    in_maps = []
    for c in range(NCORES):
        xc = x[c * BC:(c + 1) * BC, 0]  # (4096, 20, 34)
        in_maps.append({"xin": _prep_x(xc), "win": blob})
    res = run_bass_kernel_spmd(nc, in_maps, list(range(NCORES)),
                               trace=_CACHE.get("trace", False))
    _CACHE["last_res"] = res
    out = np.empty((B_TOTAL, CLS), dtype=np.float32)
    for c in range(NCORES):
        out[c * BC:(c + 1) * BC] = _unpack_out(res.results[c]["oout"])
    return out


# revision 32
# speedup vs baseline: 1.5103x; 1.0172x over previous
"""Trainium2 Bass kernel for nn_BasicRNN_42271068127787.

3-layer LSTM (input=20, hidden=6, seq=34) + FC(204->20) + log_softmax over
batch 32768, data-parallel over 8 NeuronCores (4096 rows/core).

Layout (per core), redesigned for ACT-engine throughput (the bottleneck):
  - batch 4096 -> 21 chunks x 196 cols (4116, 20 zero-pad elems)
  - gate tensors live in per-gate PSUM regions [126, 196] (126 = 21*6
    partitions), packed on a (bank, col-offset) grid so ONE activation op
    covers a gate across all 3 live layers:
      G[128, 6, 512]: bank l      = [i_l | f_l]  (cols 0:196 | 196:392)
                      bank 3+l    = [o_l | g_l]
    ACT per stage: sigmoid(i,f) x3 layers (1 op), tanh(g) (1), sigmoid(o)
    (1), tanh(c) (1) -- free sizes 1176/588/588/588 at 126 partitions vs
    the old 64-partition layout's 6240.
  - all matmul operands fp16 (1 cyc/col at any N; fp32r needs N>=256),
    weights+x+h/c states fp16; PSUM accumulation fp32.
  - single-gate matmuls: x (L0) in 4 row-passes/gate; h-inputs single-pass
    [127 or 126 rows]; bias folded via const-1.0 row 126 of the h tile.
  - DVE chain fused across layers: z = S_if*[tg|c] (1), c' = z_i+z_f (1),
    h = S_o*tanh(c') (1); fp16 2x mode.
  - FC accumulated inline over t into 2 pinned PSUM banks, 4 chunk-groups
    (6,6,6,3); fc bias as fc_b/SEQ via const row.
  - log_softmax tail on device (no max subtraction; logits are O(1)).
"""

import sys

import numpy as np

if "/opt/trn_rl_repo" not in sys.path:
    sys.path.insert(0, "/opt/trn_rl_repo")

B_TOTAL = 32768
INPUT = 20
HID = 6
SEQ = 34
CLS = 20
NCORES = 8
BC = B_TOTAL // NCORES   # 4096
NB = 21                  # batch chunks per core
BF = 196                 # batch cols per chunk
BCP = NB * BF            # 4116 padded batch per core
XCH = (6, 6, 6, 3)       # chunks per x-matmul pass
FCG = (6, 6, 6, 3)       # chunks per FC output group
# gate -> (bank, col offset) in the G PSUM tile; torch gate index
GATES = (("i", 0), ("f", 1), ("o", 3), ("g", 2))


def _gate_region(gname, l=0):
    # -> (layer-bank, half-bank) inside Gif (i,f) or Gog (o,g) PSUM tiles
    return l, (0 if gname in ("i", "o") else 1)


_CACHE = {}


# ---------------------------------------------------------------- host prep

def _build_wblob(w_ih, w_hh, b_ih, b_hh, fc_w, fc_b):
    """Pack all lhsT weight tiles into one [128, WC] fp16 blob."""
    cols = {}
    blocks = []
    cursor = 0

    def alloc(name, n):
        nonlocal cursor
        cols[name] = cursor
        arr = np.zeros((128, n), dtype=np.float32)
        blocks.append(arr)
        cursor += n
        return arr

    bsum = [b_ih[l] + b_hh[l] for l in range(3)]

    # DMA-priority order: ones + h-tiles first (stage 0 needs them), then
    # x tiles, then FC per t, then tail ones.
    a = alloc("ones", BF)
    a[:] = 1.0

    # h-input lhsT tiles [127 or 126, 126], block-diag per chunk; bias on
    # row 126 for the tiles that pair with the const-1.0 rhs row.
    def hblk(name, w, gt, bias):
        a = alloc(name, 126)
        for c in range(NB):
            a[6 * c:6 * c + 6, 6 * c:6 * c + 6] = w[gt * 6:gt * 6 + 6, :].T
        if bias is not None:
            for c in range(NB):
                a[126, 6 * c:6 * c + 6] = bias[gt * 6:gt * 6 + 6]

    for gname, gt in GATES:
        hblk("h0%s" % gname, w_hh[0], gt, bsum[0])
        hblk("a1%s" % gname, w_ih[1], gt, bsum[1])
        hblk("b1%s" % gname, w_hh[1], gt, None)
        hblk("a2%s" % gname, w_ih[2], gt, bsum[2])
        hblk("b2%s" % gname, w_hh[2], gt, None)
    # L0 x tiles: pass p covers chunks 6p..6p+ncp-1; row cc*20+k -> out col
    # 36p+cc*6+h with w_ih0[gt*6+h, k].  Out cols span the full 126-row
    # region (PE requires out base partition 0/32/64), zero elsewhere.
    for gname, gt in GATES:
        for p in range(4):
            ncp = XCH[p]
            a = alloc("x%s%d" % (gname, p), 126)
            for cc in range(ncp):
                o = 36 * p + cc * 6
                a[cc * 20:cc * 20 + 20, o:o + 6] = \
                    w_ih[0][gt * 6:gt * 6 + 6, :].T
    # FC tiles per (t, group): rows 6c+h -> col cc*20+cl
    for t in range(SEQ):
        for j in range(4):
            ncj = FCG[j]
            a = alloc("fc%d_%d" % (t, j), 20 * ncj)
            for cc in range(ncj):
                c = 6 * j + cc
                a[6 * c:6 * c + 6, cc * 20:cc * 20 + 20] = \
                    fc_w[:, t * 6:t * 6 + 6].T
                a[126, cc * 20:cc * 20 + 20] = fc_b / SEQ
    # tail reduce/broadcast ones (out cols span full region, zero elsewhere)
    for j in range(4):
        ncj = FCG[j]
        a = alloc("redK%d" % j, NB)
        for cc in range(ncj):
            a[cc * 20:cc * 20 + 20, 6 * j + cc] = 1.0
        a = alloc("redM%d" % j, 20 * ncj)
        for cc in range(ncj):
            a[6 * j + cc, cc * 20:cc * 20 + 20] = 1.0

    blob = np.concatenate(blocks, axis=1).astype(np.float16)
    return np.ascontiguousarray(blob), cols


def _prep_x(x_core):
    """(4096, 20, 34) -> [34, 120, 4, 196] fp16; pass p rows cc*20+f."""
    xp = np.zeros((BCP, INPUT, SEQ), dtype=np.float32)
    xp[:BC] = x_core
    arr = xp.reshape(NB, BF, INPUT, SEQ).transpose(3, 0, 2, 1)  # (34,21,20,196)
    a24 = np.zeros((SEQ, 24, INPUT, BF), dtype=np.float32)
    a24[:, :NB] = arr
    a24 = a24.reshape(SEQ, 4, 6 * INPUT, BF).transpose(0, 2, 1, 3)
    return np.ascontiguousarray(a24.astype(np.float16))  # (34, 120, 4, 196)


def _unpack_out(od):
    """[120, 4, 196] f32 -> (4096, 20)."""
    r = od.reshape(6, CLS, 4, BF).transpose(2, 0, 3, 1)  # (grp, cc, col, cls)
    return r.reshape(24 * BF, CLS)[:BC]


# ---------------------------------------------------------------- program

def _make_nc(wc_total, col):
    import concourse.tile as tile
    from concourse import bacc, mybir

    F = mybir.dt.float32
    H16 = mybir.dt.float16
    AF = mybir.ActivationFunctionType
    Alu = mybir.AluOpType

    nc = bacc.Bacc("TRN2", target_bir_lowering=False, debug=False)
    xd = nc.declare_dram_parameter("xin", [SEQ, 120, 4, BF], H16, isOutput=False)
    wd = nc.declare_dram_parameter("win", [128, wc_total], H16, isOutput=False)
    od = nc.declare_dram_parameter("oout", [120, 2, 2, BF], F, isOutput=True)

    with tile.TileContext(nc) as tc:
        with (
            tc.tile_pool(name="w", bufs=1) as wp,
            tc.tile_pool(name="x", bufs=4) as xp,
            tc.tile_pool(name="s", bufs=2) as sp,
            tc.tile_pool(name="st", bufs=1) as st,
            tc.tile_pool(name="g", bufs=1, space="PSUM") as gp,
            tc.tile_pool(name="fc", bufs=1, space="PSUM") as fp,
        ):
            wsb = wp.tile([128, wc_total], H16)
            # chunked weight DMA so early stages start before FC tiles land
            w_splits = [0, col["xi0"], col["fc0_0"], col["fc6_0"],
                        col["fc17_0"], wc_total]
            for a, b in zip(w_splits[:-1], w_splits[1:]):
                nc.sync.dma_start(out=wsb[:, a:b], in_=wd[:, a:b])

            def wap(name, r0, r1, c0, c1):
                c = col[name]
                return wsb[r0:r1, c + c0:c + c1]

            # persistent state, per stream (cols 98k:98k+98 of each chunk);
            # dim layout [part, layer, slot, col].  H double-buffered by
            # stage parity so FC(s) can be emitted a stage late (off the
            # critical path) while still reading h2(s).
            BS = BF // 2  # 98 cols per stream
            Hs, Tt, Sif, So, TC, Zt = [], [], [], [], [], []
            for k in range(2):
                Hs.append([st.tile([128, 3, BS], H16, tag="H%d%d" % (k, p),
                                   name="H%d%d" % (k, p)) for p in range(2)])
                Tt.append(st.tile([128, 3, 2, BS], H16, tag="T%d" % k,
                                  name="T%d" % k))   # slot 0=tanh(g), 1=c
                Sif.append(st.tile([128, 3, 2, BS], H16, tag="S%d" % k,
                                   name="S%d" % k))  # slot 0=sig_i, 1=sig_f
                So.append(st.tile([128, 3, BS], H16, tag="O%d" % k,
                                  name="O%d" % k))
                TC.append(st.tile([128, 3, BS], H16, tag="C%d" % k,
                                  name="C%d" % k))
                Zt.append(st.tile([128, 3, 2, BS], H16, tag="Z%d" % k,
                                  name="Z%d" % k))
                for p in range(2):
                    nc.vector.memset(Hs[k][p][:], 0.0)
                    for l in range(3):
                        # bias row 126 = 1.0 (engine ops can't address
                        # partition 126; SBUF->SBUF DMA can)
                        nc.sync.dma_start(out=Hs[k][p][126:127, l, :],
                                          in_=wap("ones", 126, 127, 0, BS))
                nc.vector.memset(Tt[k][:], 0.0)

            # separate PSUM tiles so WAR deps (tile-granular) don't serialize
            # o/g matmuls behind sigmoid(i,f) reads
            Gif = gp.tile([128, 3, 2, 256], F, tag="Gif", name="Gif")
            Gog = gp.tile([128, 3, 2, 256], F, tag="Gog", name="Gog")
            FCp = fp.tile([128, 2, 2, 256], F, tag="FC")
            nc.vector.memset(FCp[:], 0.0)

            def fc_region(j, r0, r1, c0=0, c1=BF):
                return FCp[r0:r1, j // 2, j % 2, c0:c1]

            def mm(out, lhsT, rhs, start, stop):
                nc.tensor.matmul(out, lhsT, rhs, start=start, stop=stop,
                                 skip_group_check=True)

            def emit_x(s_, xa, gates=("i", "f", "o", "g")):
                # x matmuls (L0): 4 chunk-passes per gate accumulating [0:126]
                for gname in gates:
                    _, hf = _gate_region(gname, 0)
                    gtile = Gif if gname in ("i", "f") else Gog
                    for p in range(4):
                        ncp = XCH[p]
                        mm(gtile[0:126, 0, hf, 0:BF],
                           wap("x%s%d" % (gname, p), 0, 20 * ncp, 0, 126),
                           xa[0:20 * ncp, p, :], start=(p == 0), stop=False)

            def stream_chain(s_, k, l0, l1):
                """One stream's per-stage recurrence on cols 98k:98k+98."""
                c0, c1 = BS * k, BS * k + BS
                Hp = Hs[k][s_ % 2]        # h(s-1) inputs
                Hn = Hs[k][(s_ + 1) % 2]  # h(s) outputs

                def hmm(gset, gtile):
                    for gname in gset:
                        for l in range(l0, l1 + 1):
                            _, hf = _gate_region(gname, l)
                            out = gtile[0:126, l, hf, c0:c1]
                            x_open = (l == 0 and s_ < SEQ)
                            if l == 0:
                                mm(out, wap("h0%s" % gname, 0, 127, 0, 126),
                                   Hp[0:127, 0, :], start=not x_open, stop=True)
                            else:
                                nm = ("a1", "b1") if l == 1 else ("a2", "b2")
                                mm(out,
                                   wap("%s%s" % (nm[0], gname), 0, 127, 0, 126),
                                   Hp[0:127, l - 1, :], start=True, stop=False)
                                mm(out,
                                   wap("%s%s" % (nm[1], gname), 0, 126, 0, 126),
                                   Hp[0:126, l, :], start=False, stop=True)

                hmm(("i", "f"), Gif)
                # sigmoid(i|f) across live layers: one fused op
                nc.scalar.activation(out=Sif[k][0:126, l0:l1 + 1, :, :],
                                     in_=Gif[0:126, l0:l1 + 1, 0:2, c0:c1],
                                     func=AF.Sigmoid)
                hmm(("o", "g"), Gog)
                # z_f = sig_f*c right after sigmoid (no tanh_g dep)
                nc.vector.tensor_mul(out=Zt[k][0:126, l0:l1 + 1, 1:2, :],
                                     in0=Sif[k][0:126, l0:l1 + 1, 1:2, :],
                                     in1=Tt[k][0:126, l0:l1 + 1, 1:2, :])
                # tanh(g) -> T slot 0; sigmoid(o) -> So
                nc.scalar.activation(out=Tt[k][0:126, l0:l1 + 1, 0:1, :],
                                     in_=Gog[0:126, l0:l1 + 1, 1:2, c0:c1],
                                     func=AF.Tanh)
                nc.scalar.activation(out=So[k][0:126, l0:l1 + 1, :],
                                     in_=Gog[0:126, l0:l1 + 1, 0, c0:c1],
                                     func=AF.Sigmoid)
                # z_i = sig_i*tanh_g; c' = z_i + z_f; h = sig_o*tanh(c')
                nc.vector.tensor_mul(out=Zt[k][0:126, l0:l1 + 1, 0:1, :],
                                     in0=Sif[k][0:126, l0:l1 + 1, 0:1, :],
                                     in1=Tt[k][0:126, l0:l1 + 1, 0:1, :])
                nc.vector.tensor_add(out=Tt[k][0:126, l0:l1 + 1, 1:2, :],
                                     in0=Zt[k][0:126, l0:l1 + 1, 0:1, :],
                                     in1=Zt[k][0:126, l0:l1 + 1, 1:2, :])
                nc.scalar.activation(out=TC[k][0:126, l0:l1 + 1, :],
                                     in_=Tt[k][0:126, l0:l1 + 1, 1, :],
                                     func=AF.Tanh)
                nc.vector.tensor_mul(out=Hn[0:126, l0:l1 + 1, :],
                                     in0=So[k][0:126, l0:l1 + 1, :],
                                     in1=TC[k][0:126, l0:l1 + 1, :])

            xtiles = {}

            def emit_fc(t2):
                for k in range(2):
                    for j in range(4):
                        ncj = FCG[j]
                        mm(fc_region(j, 0, 20 * ncj, BS * k, BS * k + BS),
                           wap("fc%d_%d" % (t2, j), 0, 127, 0, 20 * ncj),
                           Hs[k][(t2 + 3) % 2][0:127, 2, :],
                           start=(t2 == 0), stop=(t2 == SEQ - 1))

            for s_ in range(SEQ + 2):
                l0, l1 = max(0, s_ - (SEQ - 1)), min(2, s_)
                # FC first in PE FIFO: it is always ready (double-buffered h)
                # and runs while the h-matmuls still wait on h(s-1)
                if 0 <= s_ - 4 < SEQ:
                    emit_fc(s_ - 4)
                if s_ == 0:
                    for t in (0, 1):
                        xa = xp.tile([120, 4, BF], H16, tag="xa",
                                     name="xa%d" % t)
                        nc.sync.dma_start(out=xa[:], in_=xd[t])
                        xtiles[t] = xa
                    emit_x(0, xtiles[0])
                stream_chain(s_, 0, l0, l1)
                stream_chain(s_, 1, l0, l1)
                if s_ + 1 < SEQ:
                    emit_x(s_ + 1, xtiles[s_ + 1])
                if s_ + 2 < SEQ:
                    xa = xp.tile([120, 4, BF], H16, tag="xa",
                                 name="xa%d" % (s_ + 2))
                    nc.sync.dma_start(out=xa[:], in_=xd[s_ + 2])
                    xtiles[s_ + 2] = xa
            # flush the last FC steps (t2 emitted at s_ = t2+4 > SEQ+1)
            for t2 in (SEQ - 2, SEQ - 1):
                emit_fc(t2)

            # ---- log_softmax tail (logits O(1); skip max subtraction)
            Lsb = sp.tile([128, 2, 2, BF], F, tag="Lsb")
            Esb = sp.tile([128, 2, 2, BF], H16, tag="Esb")
            nc.scalar.activation(out=Lsb[0:120, :, :, :],
                                 in_=FCp[0:120, 0:2, 0:2, 0:BF],
                                 func=AF.Identity)
            nc.scalar.activation(out=Esb[0:120, :, :, :],
                                 in_=FCp[0:120, 0:2, 0:2, 0:BF], func=AF.Exp)
            s_ps = gp.tile([21, BF], F, tag="Gif", name="Gsum")
            for j in range(4):
                ncj = FCG[j]
                mm(s_ps[0:21, :], wap("redK%d" % j, 0, 20 * ncj, 0, NB),
                   Esb[0:20 * ncj, j // 2, j % 2, :],
                   start=(j == 0), stop=(j == 3))
            lnz = sp.tile([21, BF], H16, tag="lnz")
            nc.scalar.activation(out=lnz[:], in_=s_ps[0:21, :], func=AF.Ln)
            bc = gp.tile([128, 2, 2, 256], F, tag="Gog", name="Gbc")
            for j in range(4):
                ncj = FCG[j]
                mm(bc[0:20 * ncj, j // 2, j % 2, 0:BF],
                   wap("redM%d" % j, 0, NB, 0, 20 * ncj),
                   lnz[0:21, :], start=True, stop=True)
            Osb = sp.tile([128, 2, 2, BF], F, tag="Osb")
            nc.vector.scalar_tensor_tensor(
                out=Osb[0:120, :, :, :], in0=bc[0:120, 0:2, 0:2, 0:BF],
                scalar=-1.0, in1=Lsb[0:120, :, :, :],
                op0=Alu.mult, op1=Alu.add)
            nc.sync.dma_start(out=od[:], in_=Osb[0:120, :, :, :])
    nc.compile()
    return nc


def _get_program(inputs):
    w_ih = [inputs["w_ih%d" % l] for l in range(3)]
    w_hh = [inputs["w_hh%d" % l] for l in range(3)]
    b_ih = [inputs["b_ih%d" % l] for l in range(3)]
    b_hh = [inputs["b_hh%d" % l] for l in range(3)]
    blob, col = _build_wblob(w_ih, w_hh, b_ih, b_hh,
                             inputs["fc_w"], inputs["fc_b"])
    if "nc1" not in _CACHE:
        _CACHE["nc1"] = _make_nc(blob.shape[1], col)
    return _CACHE["nc1"], blob


def kernel(**inputs):
    from concourse.bass_utils import run_bass_kernel_spmd

    nc, blob = _get_program(inputs)
    x = np.asarray(inputs["x"], dtype=np.float32)
    in_maps = []
    for c in range(NCORES):
        xc = x[c * BC:(c + 1) * BC, 0]  # (4096, 20, 34)
        in_maps.append({"xin": _prep_x(xc), "win": blob})
    res = run_bass_kernel_spmd(nc, in_maps, list(range(NCORES)),
                               trace=_CACHE.get("trace", False))
    _CACHE["last_res"] = res
    out = np.empty((B_TOTAL, CLS), dtype=np.float32)
    for c in range(NCORES):
        out[c * BC:(c + 1) * BC] = _unpack_out(res.results[c]["oout"])
    return out


# revision 39
# speedup vs baseline: 1.6375x; 1.0842x over previous
"""Trainium2 Bass kernel for nn_BasicRNN_42271068127787.

3-layer LSTM (input=20, hidden=6, seq=34) + FC(204->20) + log_softmax over
batch 32768, data-parallel over 8 NeuronCores (4096 rows/core).

Layout (per core), redesigned for ACT-engine throughput (the bottleneck):
  - batch 4096 -> 21 chunks x 196 cols (4116, 20 zero-pad elems)
  - gate tensors live in per-gate PSUM regions [126, 196] (126 = 21*6
    partitions), packed on a (bank, col-offset) grid so ONE activation op
    covers a gate across all 3 live layers:
      G[128, 6, 512]: bank l      = [i_l | f_l]  (cols 0:196 | 196:392)
                      bank 3+l    = [o_l | g_l]
    ACT per stage: sigmoid(i,f) x3 layers (1 op), tanh(g) (1), sigmoid(o)
    (1), tanh(c) (1) -- free sizes 1176/588/588/588 at 126 partitions vs
    the old 64-partition layout's 6240.
  - all matmul operands fp16 (1 cyc/col at any N; fp32r needs N>=256),
    weights+x+h/c states fp16; PSUM accumulation fp32.
  - single-gate matmuls: x (L0) in 4 row-passes/gate; h-inputs single-pass
    [127 or 126 rows]; bias folded via const-1.0 row 126 of the h tile.
  - DVE chain fused across layers: z = S_if*[tg|c] (1), c' = z_i+z_f (1),
    h = S_o*tanh(c') (1); fp16 2x mode.
  - FC accumulated inline over t into 2 pinned PSUM banks, 4 chunk-groups
    (6,6,6,3); fc bias as fc_b/SEQ via const row.
  - log_softmax tail on device (no max subtraction; logits are O(1)).
"""

import sys

import numpy as np

if "/opt/trn_rl_repo" not in sys.path:
    sys.path.insert(0, "/opt/trn_rl_repo")

B_TOTAL = 32768
INPUT = 20
HID = 6
SEQ = 34
CLS = 20
NCORES = 8
BC = B_TOTAL // NCORES   # 4096
NB = 21                  # batch chunks per core
BF = 196                 # batch cols per chunk
BCP = NB * BF            # 4116 padded batch per core
XCH = (6, 6, 6, 3)       # chunks per x-matmul pass
FCG = (6, 6, 6, 3)       # chunks per FC output group
# gate -> (bank, col offset) in the G PSUM tile; torch gate index
GATES = (("i", 0), ("f", 1), ("o", 3), ("g", 2))


def _gate_region(gname, l=0):
    # -> (layer-bank, half-bank) inside Gif (i,f) or Gog (o,g) PSUM tiles
    return l, (0 if gname in ("i", "o") else 1)


_CACHE = {}


# ---------------------------------------------------------------- host prep

def _build_wblob(w_ih, w_hh, b_ih, b_hh, fc_w, fc_b):
    """Pack all lhsT weight tiles into one [128, WC] fp16 blob."""
    cols = {}
    blocks = []
    cursor = 0

    def alloc(name, n):
        nonlocal cursor
        cols[name] = cursor
        arr = np.zeros((128, n), dtype=np.float32)
        blocks.append(arr)
        cursor += n
        return arr

    bsum = [b_ih[l] + b_hh[l] for l in range(3)]

    # DMA-priority order: ones + h-tiles first (stage 0 needs them), then
    # x tiles, then FC per t, then tail ones.
    a = alloc("ones", BF)
    a[:] = 1.0

    # h-input lhsT tiles [127 or 126, 126], block-diag per chunk; bias on
    # row 126 for the tiles that pair with the const-1.0 rhs row.
    def hblk(name, w, gt, bias):
        a = alloc(name, 126)
        for c in range(NB):
            a[6 * c:6 * c + 6, 6 * c:6 * c + 6] = w[gt * 6:gt * 6 + 6, :].T
        if bias is not None:
            for c in range(NB):
                a[126, 6 * c:6 * c + 6] = bias[gt * 6:gt * 6 + 6]

    for gname, gt in GATES:
        hblk("h0%s" % gname, w_hh[0], gt, bsum[0])
        hblk("a1%s" % gname, w_ih[1], gt, bsum[1])
        hblk("b1%s" % gname, w_hh[1], gt, None)
        hblk("a2%s" % gname, w_ih[2], gt, bsum[2])
        hblk("b2%s" % gname, w_hh[2], gt, None)
    # L0 x tiles: pass p covers chunks 6p..6p+ncp-1; row cc*20+k -> out col
    # 36p+cc*6+h with w_ih0[gt*6+h, k].  Out cols span the full 126-row
    # region (PE requires out base partition 0/32/64), zero elsewhere.
    for gname, gt in GATES:
        for p in range(4):
            ncp = XCH[p]
            a = alloc("x%s%d" % (gname, p), 126)
            for cc in range(ncp):
                o = 36 * p + cc * 6
                a[cc * 20:cc * 20 + 20, o:o + 6] = \
                    w_ih[0][gt * 6:gt * 6 + 6, :].T
    # FC tiles per (t, group): rows 6c+h -> col cc*20+cl
    for t in range(SEQ):
        for j in range(4):
            ncj = FCG[j]
            a = alloc("fc%d_%d" % (t, j), 20 * ncj)
            for cc in range(ncj):
                c = 6 * j + cc
                a[6 * c:6 * c + 6, cc * 20:cc * 20 + 20] = \
                    fc_w[:, t * 6:t * 6 + 6].T
                a[126, cc * 20:cc * 20 + 20] = fc_b / SEQ
    # tail reduce/broadcast ones (out cols span full region, zero elsewhere)
    for j in range(4):
        ncj = FCG[j]
        a = alloc("redK%d" % j, NB)
        for cc in range(ncj):
            a[cc * 20:cc * 20 + 20, 6 * j + cc] = 1.0
        a = alloc("redM%d" % j, 20 * ncj)
        for cc in range(ncj):
            a[6 * j + cc, cc * 20:cc * 20 + 20] = 1.0

    blob = np.concatenate(blocks, axis=1).astype(np.float16)
    return np.ascontiguousarray(blob), cols


def _prep_x(x_core):
    """(4096, 20, 34) -> [34, 120, 4, 196] fp16; pass p rows cc*20+f."""
    xp = np.zeros((BCP, INPUT, SEQ), dtype=np.float32)
    xp[:BC] = x_core
    arr = xp.reshape(NB, BF, INPUT, SEQ).transpose(3, 0, 2, 1)  # (34,21,20,196)
    a24 = np.zeros((SEQ, 24, INPUT, BF), dtype=np.float32)
    a24[:, :NB] = arr
    a24 = a24.reshape(SEQ, 4, 6 * INPUT, BF).transpose(0, 2, 1, 3)
    return np.ascontiguousarray(a24.astype(np.float16))  # (34, 120, 4, 196)


def _unpack_out(od):
    """[120, 4, 196] f32 -> (4096, 20)."""
    r = od.reshape(6, CLS, 4, BF).transpose(2, 0, 3, 1)  # (grp, cc, col, cls)
    return r.reshape(24 * BF, CLS)[:BC]


# ---------------------------------------------------------------- program

def _make_nc(wc_total, col):
    import concourse.tile as tile
    from concourse import bacc, mybir

    F = mybir.dt.float32
    H16 = mybir.dt.float16
    AF = mybir.ActivationFunctionType
    Alu = mybir.AluOpType

    nc = bacc.Bacc("TRN2", target_bir_lowering=False, debug=False)
    xd = nc.declare_dram_parameter("xin", [SEQ, 120, 4, BF], H16, isOutput=False)
    wd = nc.declare_dram_parameter("win", [128, wc_total], H16, isOutput=False)
    od = nc.declare_dram_parameter("oout", [120, 2, 2, BF], F, isOutput=True)

    with tile.TileContext(nc) as tc:
        with (
            tc.tile_pool(name="w", bufs=1) as wp,
            tc.tile_pool(name="x", bufs=4) as xp,
            tc.tile_pool(name="s", bufs=2) as sp,
            tc.tile_pool(name="st", bufs=1) as st,
            tc.tile_pool(name="g", bufs=1, space="PSUM") as gp,
            tc.tile_pool(name="fc", bufs=1, space="PSUM") as fp,
        ):
            wsb = wp.tile([128, wc_total], H16)
            # chunked weight DMA so early stages start before FC tiles land
            w_splits = [0, col["xi0"], col["fc0_0"], col["fc6_0"],
                        col["fc17_0"], wc_total]
            for a, b in zip(w_splits[:-1], w_splits[1:]):
                nc.sync.dma_start(out=wsb[:, a:b], in_=wd[:, a:b])

            def wap(name, r0, r1, c0, c1):
                c = col[name]
                return wsb[r0:r1, c + c0:c + c1]

            # persistent state, per stream (cols 98k:98k+98 of each chunk);
            # dim layout [part, layer, slot, col].  H double-buffered by
            # stage parity so FC(s) can be emitted a stage late (off the
            # critical path) while still reading h2(s).
            BS = BF // 2  # 98 cols per stream
            Hs, Tt, Sif, So, TC, Zt = [], [], [], [], [], []
            for k in range(2):
                Hs.append([st.tile([128, 3, BS], H16, tag="H%d%d" % (k, p),
                                   name="H%d%d" % (k, p)) for p in range(2)])
                Tt.append(st.tile([128, 3, 2, BS], H16, tag="T%d" % k,
                                  name="T%d" % k))   # slot 0=tanh(g), 1=c
                Sif.append(st.tile([128, 3, 2, BS], H16, tag="S%d" % k,
                                   name="S%d" % k))  # slot 0=sig_i, 1=sig_f
                So.append(st.tile([128, 3, BS], H16, tag="O%d" % k,
                                  name="O%d" % k))
                TC.append(st.tile([128, 3, BS], H16, tag="C%d" % k,
                                  name="C%d" % k))
                Zt.append(st.tile([128, 3, 2, BS], H16, tag="Z%d" % k,
                                  name="Z%d" % k))
                for p in range(2):
                    # bias row: engine ops need 32-aligned partition bases,
                    # so write 1.0 to 96:128 then re-zero 96:126
                    nc.vector.memset(Hs[k][p][0:96, :, :], 0.0)
                    nc.vector.memset(Hs[k][p][96:128, :, :], 1.0)
                    nc.vector.memset(Hs[k][p][96:126, :, :], 0.0)
                nc.vector.memset(Tt[k][:], 0.0)

            # separate PSUM tiles so WAR deps (tile-granular) don't serialize
            # o/g matmuls behind sigmoid(i,f) reads
            Gif = gp.tile([128, 3, 2, 256], F, tag="Gif", name="Gif")
            Gog = gp.tile([128, 3, 2, 256], F, tag="Gog", name="Gog")
            FCp = fp.tile([128, 2, 2, 256], F, tag="FC")
            nc.vector.memset(FCp[:], 0.0)

            def fc_region(j, r0, r1, c0=0, c1=BF):
                return FCp[r0:r1, j // 2, j % 2, c0:c1]

            def mm(out, lhsT, rhs, start, stop):
                nc.tensor.matmul(out, lhsT, rhs, start=start, stop=stop,
                                 skip_group_check=True)

            def emit_x(s_, xa, gates=("i", "f", "o", "g")):
                # x matmuls (L0): 4 chunk-passes per gate accumulating [0:126]
                for gname in gates:
                    _, hf = _gate_region(gname, 0)
                    gtile = Gif if gname in ("i", "f") else Gog
                    for p in range(4):
                        ncp = XCH[p]
                        mm(gtile[0:126, 0, hf, 0:BF],
                           wap("x%s%d" % (gname, p), 0, 20 * ncp, 0, 126),
                           xa[0:20 * ncp, p, :], start=(p == 0), stop=False)

            def stream_chain(s_, k, l0, l1):
                """One stream's per-stage recurrence on cols 98k:98k+98."""
                c0, c1 = BS * k, BS * k + BS
                Hp = Hs[k][s_ % 2]        # h(s-1) inputs
                Hn = Hs[k][(s_ + 1) % 2]  # h(s) outputs

                def hmm(gset, gtile):
                    for gname in gset:
                        for l in range(l0, l1 + 1):
                            _, hf = _gate_region(gname, l)
                            out = gtile[0:126, l, hf, c0:c1]
                            x_open = (l == 0 and s_ < SEQ)
                            if l == 0:
                                mm(out, wap("h0%s" % gname, 0, 127, 0, 126),
                                   Hp[0:127, 0, :], start=not x_open, stop=True)
                            else:
                                nm = ("a1", "b1") if l == 1 else ("a2", "b2")
                                mm(out,
                                   wap("%s%s" % (nm[0], gname), 0, 127, 0, 126),
                                   Hp[0:127, l - 1, :], start=True, stop=False)
                                mm(out,
                                   wap("%s%s" % (nm[1], gname), 0, 126, 0, 126),
                                   Hp[0:126, l, :], start=False, stop=True)

                hmm(("i", "f"), Gif)
                hmm(("o", "g"), Gog)
                # sigmoid(i|f) across live layers: one fused op
                nc.scalar.activation(out=Sif[k][0:126, l0:l1 + 1, :, :],
                                     in_=Gif[0:126, l0:l1 + 1, 0:2, c0:c1],
                                     func=AF.Sigmoid)
                # z_f = sig_f*c right after sigmoid (no tanh_g dep)
                nc.vector.tensor_mul(out=Zt[k][0:126, l0:l1 + 1, 1:2, :],
                                     in0=Sif[k][0:126, l0:l1 + 1, 1:2, :],
                                     in1=Tt[k][0:126, l0:l1 + 1, 1:2, :])
                # tanh(g) -> T slot 0; sigmoid(o) -> So
                nc.scalar.activation(out=Tt[k][0:126, l0:l1 + 1, 0:1, :],
                                     in_=Gog[0:126, l0:l1 + 1, 1:2, c0:c1],
                                     func=AF.Tanh)
                nc.scalar.activation(out=So[k][0:126, l0:l1 + 1, :],
                                     in_=Gog[0:126, l0:l1 + 1, 0, c0:c1],
                                     func=AF.Sigmoid)
                # z_i = sig_i*tanh_g; c' = z_i + z_f; h = sig_o*tanh(c')
                nc.vector.tensor_mul(out=Zt[k][0:126, l0:l1 + 1, 0:1, :],
                                     in0=Sif[k][0:126, l0:l1 + 1, 0:1, :],
                                     in1=Tt[k][0:126, l0:l1 + 1, 0:1, :])
                nc.vector.tensor_add(out=Tt[k][0:126, l0:l1 + 1, 1:2, :],
                                     in0=Zt[k][0:126, l0:l1 + 1, 0:1, :],
                                     in1=Zt[k][0:126, l0:l1 + 1, 1:2, :])
                nc.scalar.activation(out=TC[k][0:126, l0:l1 + 1, :],
                                     in_=Tt[k][0:126, l0:l1 + 1, 1, :],
                                     func=AF.Tanh)
                nc.vector.tensor_mul(out=Hn[0:126, l0:l1 + 1, :],
                                     in0=So[k][0:126, l0:l1 + 1, :],
                                     in1=TC[k][0:126, l0:l1 + 1, :])

            xtiles = {}

            def emit_fc(t2):
                for k in range(2):
                    for j in range(4):
                        ncj = FCG[j]
                        mm(fc_region(j, 0, 20 * ncj, BS * k, BS * k + BS),
                           wap("fc%d_%d" % (t2, j), 0, 127, 0, 20 * ncj),
                           Hs[k][(t2 + 3) % 2][0:127, 2, :],
                           start=(t2 == 0), stop=(t2 == SEQ - 1))

            for s_ in range(SEQ + 2):
                l0, l1 = max(0, s_ - (SEQ - 1)), min(2, s_)
                # FC first in PE FIFO: it is always ready (double-buffered h)
                # and runs while the h-matmuls still wait on h(s-1)
                if 0 <= s_ - 4 < SEQ:
                    emit_fc(s_ - 4)
                if s_ == 0:
                    for t in (0, 1):
                        xa = xp.tile([120, 4, BF], H16, tag="xa",
                                     name="xa%d" % t)
                        nc.gpsimd.dma_start(out=xa[:], in_=xd[t])
                        xtiles[t] = xa
                    emit_x(0, xtiles[0])
                stream_chain(s_, 0, l0, l1)
                stream_chain(s_, 1, l0, l1)
                if s_ + 1 < SEQ:
                    emit_x(s_ + 1, xtiles[s_ + 1])
                if s_ + 2 < SEQ:
                    xa = xp.tile([120, 4, BF], H16, tag="xa",
                                 name="xa%d" % (s_ + 2))
                    nc.gpsimd.dma_start(out=xa[:], in_=xd[s_ + 2])
                    xtiles[s_ + 2] = xa
            # flush the last FC steps (t2 emitted at s_ = t2+4 > SEQ+1)
            for t2 in (SEQ - 2, SEQ - 1):
                emit_fc(t2)

            # ---- log_softmax tail (logits O(1); skip max subtraction)
            Lsb = sp.tile([128, 2, 2, BF], F, tag="Lsb")
            Esb = sp.tile([128, 2, 2, BF], H16, tag="Esb")
            nc.scalar.activation(out=Lsb[0:120, :, :, :],
                                 in_=FCp[0:120, 0:2, 0:2, 0:BF],
                                 func=AF.Identity)
            nc.scalar.activation(out=Esb[0:120, :, :, :],
                                 in_=FCp[0:120, 0:2, 0:2, 0:BF], func=AF.Exp)
            s_ps = gp.tile([21, BF], F, tag="Gif", name="Gsum")
            for j in range(4):
                ncj = FCG[j]
                mm(s_ps[0:21, :], wap("redK%d" % j, 0, 20 * ncj, 0, NB),
                   Esb[0:20 * ncj, j // 2, j % 2, :],
                   start=(j == 0), stop=(j == 3))
            lnz = sp.tile([21, BF], H16, tag="lnz")
            nc.scalar.activation(out=lnz[:], in_=s_ps[0:21, :], func=AF.Ln)
            bc = gp.tile([128, 2, 2, 256], F, tag="Gog", name="Gbc")
            for j in range(4):
                ncj = FCG[j]
                mm(bc[0:20 * ncj, j // 2, j % 2, 0:BF],
                   wap("redM%d" % j, 0, NB, 0, 20 * ncj),
                   lnz[0:21, :], start=True, stop=True)
            Osb = sp.tile([128, 2, 2, BF], F, tag="Osb")
            nc.vector.scalar_tensor_tensor(
                out=Osb[0:120, :, :, :], in0=bc[0:120, 0:2, 0:2, 0:BF],
                scalar=-1.0, in1=Lsb[0:120, :, :, :],
                op0=Alu.mult, op1=Alu.add)
            nc.sync.dma_start(out=od[:], in_=Osb[0:120, :, :, :])
    nc.compile()
    return nc


def _get_program(inputs):
    w_ih = [inputs["w_ih%d" % l] for l in range(3)]
    w_hh = [inputs["w_hh%d" % l] for l in range(3)]
    b_ih = [inputs["b_ih%d" % l] for l in range(3)]
    b_hh = [inputs["b_hh%d" % l] for l in range(3)]
    blob, col = _build_wblob(w_ih, w_hh, b_ih, b_hh,
                             inputs["fc_w"], inputs["fc_b"])
    if "nc1" not in _CACHE:
        _CACHE["nc1"] = _make_nc(blob.shape[1], col)
    return _CACHE["nc1"], blob


def kernel(**inputs):
    from concourse.bass_utils import run_bass_kernel_spmd

    nc, blob = _get_program(inputs)
    x = np.asarray(inputs["x"], dtype=np.float32)
    in_maps = []
    for c in range(NCORES):
        xc = x[c * BC:(c + 1) * BC, 0]  # (4096, 20, 34)
        in_maps.append({"xin": _prep_x(xc), "win": blob})
    res = run_bass_kernel_spmd(nc, in_maps, list(range(NCORES)),
                               trace=_CACHE.get("trace", False))
    _CACHE["last_res"] = res
    out = np.empty((B_TOTAL, CLS), dtype=np.float32)
    for c in range(NCORES):
        out[c * BC:(c + 1) * BC] = _unpack_out(res.results[c]["oout"])
    return out


# revision 40
# speedup vs baseline: 1.6695x; 1.0195x over previous
"""Trainium2 Bass kernel for nn_BasicRNN_42271068127787.

3-layer LSTM (input=20, hidden=6, seq=34) + FC(204->20) + log_softmax over
batch 32768, data-parallel over 8 NeuronCores (4096 rows/core).

Layout (per core), redesigned for ACT-engine throughput (the bottleneck):
  - batch 4096 -> 21 chunks x 196 cols (4116, 20 zero-pad elems)
  - gate tensors live in per-gate PSUM regions [126, 196] (126 = 21*6
    partitions), packed on a (bank, col-offset) grid so ONE activation op
    covers a gate across all 3 live layers:
      G[128, 6, 512]: bank l      = [i_l | f_l]  (cols 0:196 | 196:392)
                      bank 3+l    = [o_l | g_l]
    ACT per stage: sigmoid(i,f) x3 layers (1 op), tanh(g) (1), sigmoid(o)
    (1), tanh(c) (1) -- free sizes 1176/588/588/588 at 126 partitions vs
    the old 64-partition layout's 6240.
  - all matmul operands fp16 (1 cyc/col at any N; fp32r needs N>=256),
    weights+x+h/c states fp16; PSUM accumulation fp32.
  - single-gate matmuls: x (L0) in 4 row-passes/gate; h-inputs single-pass
    [127 or 126 rows]; bias folded via const-1.0 row 126 of the h tile.
  - DVE chain fused across layers: z = S_if*[tg|c] (1), c' = z_i+z_f (1),
    h = S_o*tanh(c') (1); fp16 2x mode.
  - FC accumulated inline over t into 2 pinned PSUM banks, 4 chunk-groups
    (6,6,6,3); fc bias as fc_b/SEQ via const row.
  - log_softmax tail on device (no max subtraction; logits are O(1)).
"""

import sys

import numpy as np

if "/opt/trn_rl_repo" not in sys.path:
    sys.path.insert(0, "/opt/trn_rl_repo")

B_TOTAL = 32768
INPUT = 20
HID = 6
SEQ = 34
CLS = 20
NCORES = 8
BC = B_TOTAL // NCORES   # 4096
NB = 21                  # batch chunks per core
BF = 196                 # batch cols per chunk
BCP = NB * BF            # 4116 padded batch per core
XCH = (6, 6, 6, 3)       # chunks per x-matmul pass
FCG = (6, 6, 6, 3)       # chunks per FC output group
# gate -> (bank, col offset) in the G PSUM tile; torch gate index
GATES = (("i", 0), ("f", 1), ("o", 3), ("g", 2))


def _gate_region(gname, l=0):
    # -> (layer-bank, half-bank) inside Gif (i,f) or Gog (o,g) PSUM tiles
    return l, (0 if gname in ("i", "o") else 1)


_CACHE = {}


# ---------------------------------------------------------------- host prep

def _build_wblob(w_ih, w_hh, b_ih, b_hh, fc_w, fc_b):
    """Pack all lhsT weight tiles into one [128, WC] fp16 blob."""
    cols = {}
    blocks = []
    cursor = 0

    def alloc(name, n):
        nonlocal cursor
        cols[name] = cursor
        arr = np.zeros((128, n), dtype=np.float32)
        blocks.append(arr)
        cursor += n
        return arr

    bsum = [b_ih[l] + b_hh[l] for l in range(3)]

    # DMA-priority order: ones + h-tiles first (stage 0 needs them), then
    # x tiles, then FC per t, then tail ones.
    a = alloc("ones", BF)
    a[:] = 1.0

    # L0 x tiles: pass p covers chunks 6p..6p+ncp-1; row cc*20+k -> out col
    # 36p+cc*6+h with w_ih0[gt*6+h, k].  Out cols span the full 126-row
    # region (PE requires out base partition 0/32/64), zero elsewhere.
    for gname, gt in GATES:
        for p in range(4):
            ncp = XCH[p]
            a = alloc("x%s%d" % (gname, p), 126)
            for cc in range(ncp):
                o = 36 * p + cc * 6
                a[cc * 20:cc * 20 + 20, o:o + 6] = \
                    w_ih[0][gt * 6:gt * 6 + 6, :].T
    # h-input lhsT tiles [127 or 126, 126], block-diag per chunk; bias on
    # row 126 for the tiles that pair with the const-1.0 rhs row.
    def hblk(name, w, gt, bias):
        a = alloc(name, 126)
        for c in range(NB):
            a[6 * c:6 * c + 6, 6 * c:6 * c + 6] = w[gt * 6:gt * 6 + 6, :].T
        if bias is not None:
            for c in range(NB):
                a[126, 6 * c:6 * c + 6] = bias[gt * 6:gt * 6 + 6]

    for gname, gt in GATES:
        hblk("h0%s" % gname, w_hh[0], gt, bsum[0])
        hblk("a1%s" % gname, w_ih[1], gt, bsum[1])
        hblk("b1%s" % gname, w_hh[1], gt, None)
        hblk("a2%s" % gname, w_ih[2], gt, bsum[2])
        hblk("b2%s" % gname, w_hh[2], gt, None)
    # FC tiles per (t, group): rows 6c+h -> col cc*20+cl
    for t in range(SEQ):
        for j in range(4):
            ncj = FCG[j]
            a = alloc("fc%d_%d" % (t, j), 20 * ncj)
            for cc in range(ncj):
                c = 6 * j + cc
                a[6 * c:6 * c + 6, cc * 20:cc * 20 + 20] = \
                    fc_w[:, t * 6:t * 6 + 6].T
                a[126, cc * 20:cc * 20 + 20] = fc_b / SEQ
    # tail reduce/broadcast ones (out cols span full region, zero elsewhere)
    for j in range(4):
        ncj = FCG[j]
        a = alloc("redK%d" % j, NB)
        for cc in range(ncj):
            a[cc * 20:cc * 20 + 20, 6 * j + cc] = 1.0
        a = alloc("redM%d" % j, 20 * ncj)
        for cc in range(ncj):
            a[6 * j + cc, cc * 20:cc * 20 + 20] = 1.0

    blob = np.concatenate(blocks, axis=1).astype(np.float16)
    return np.ascontiguousarray(blob), cols


def _prep_x(x_core):
    """(4096, 20, 34) -> [34, 120, 4, 196] fp16; pass p rows cc*20+f."""
    xp = np.zeros((BCP, INPUT, SEQ), dtype=np.float32)
    xp[:BC] = x_core
    arr = xp.reshape(NB, BF, INPUT, SEQ).transpose(3, 0, 2, 1)  # (34,21,20,196)
    a24 = np.zeros((SEQ, 24, INPUT, BF), dtype=np.float32)
    a24[:, :NB] = arr
    a24 = a24.reshape(SEQ, 4, 6 * INPUT, BF).transpose(0, 2, 1, 3)
    return np.ascontiguousarray(a24.astype(np.float16))  # (34, 120, 4, 196)


def _unpack_out(od):
    """[120, 4, 196] f32 -> (4096, 20)."""
    r = od.reshape(6, CLS, 4, BF).transpose(2, 0, 3, 1)  # (grp, cc, col, cls)
    return r.reshape(24 * BF, CLS)[:BC]


# ---------------------------------------------------------------- program

def _make_nc(wc_total, col):
    import concourse.tile as tile
    from concourse import bacc, mybir

    F = mybir.dt.float32
    H16 = mybir.dt.float16
    AF = mybir.ActivationFunctionType
    Alu = mybir.AluOpType

    nc = bacc.Bacc("TRN2", target_bir_lowering=False, debug=False)
    xd = nc.declare_dram_parameter("xin", [SEQ, 120, 4, BF], H16, isOutput=False)
    wd = nc.declare_dram_parameter("win", [128, wc_total], H16, isOutput=False)
    od = nc.declare_dram_parameter("oout", [120, 2, 2, BF], F, isOutput=True)

    with tile.TileContext(nc) as tc:
        with (
            tc.tile_pool(name="w", bufs=1) as wp,
            tc.tile_pool(name="x", bufs=4) as xp,
            tc.tile_pool(name="s", bufs=2) as sp,
            tc.tile_pool(name="st", bufs=1) as st,
            tc.tile_pool(name="g", bufs=1, space="PSUM") as gp,
            tc.tile_pool(name="fc", bufs=1, space="PSUM") as fp,
        ):
            wsb = wp.tile([128, wc_total], H16)
            # chunked weight DMA so early stages start before FC tiles land
            w_splits = [0, col["h0i"], col["fc0_0"], col["fc6_0"],
                        col["fc17_0"], wc_total]
            for a, b in zip(w_splits[:-1], w_splits[1:]):
                nc.sync.dma_start(out=wsb[:, a:b], in_=wd[:, a:b])

            def wap(name, r0, r1, c0, c1):
                c = col[name]
                return wsb[r0:r1, c + c0:c + c1]

            # persistent state, per stream (cols 98k:98k+98 of each chunk);
            # dim layout [part, layer, slot, col].  H double-buffered by
            # stage parity so FC(s) can be emitted a stage late (off the
            # critical path) while still reading h2(s).
            BS = BF // 2  # 98 cols per stream
            Hs, Tt, Sif, So, TC, Zt = [], [], [], [], [], []
            for k in range(2):
                Hs.append([st.tile([128, 3, BS], H16, tag="H%d%d" % (k, p),
                                   name="H%d%d" % (k, p)) for p in range(2)])
                Tt.append(st.tile([128, 3, 2, BS], H16, tag="T%d" % k,
                                  name="T%d" % k))   # slot 0=tanh(g), 1=c
                Sif.append(st.tile([128, 3, 2, BS], H16, tag="S%d" % k,
                                   name="S%d" % k))  # slot 0=sig_i, 1=sig_f
                So.append(st.tile([128, 3, BS], H16, tag="O%d" % k,
                                  name="O%d" % k))
                TC.append(st.tile([128, 3, BS], H16, tag="C%d" % k,
                                  name="C%d" % k))
                Zt.append(st.tile([128, 3, 2, BS], H16, tag="Z%d" % k,
                                  name="Z%d" % k))
                for p in range(2):
                    # bias row: engine ops need 32-aligned partition bases,
                    # so write 1.0 to 96:128 then re-zero 96:126
                    nc.vector.memset(Hs[k][p][0:96, :, :], 0.0)
                    nc.vector.memset(Hs[k][p][96:128, :, :], 1.0)
                    nc.vector.memset(Hs[k][p][96:126, :, :], 0.0)
                nc.vector.memset(Tt[k][:], 0.0)

            # separate PSUM tiles so WAR deps (tile-granular) don't serialize
            # o/g matmuls behind sigmoid(i,f) reads
            Gif = gp.tile([128, 3, 2, 256], F, tag="Gif", name="Gif")
            Gog = gp.tile([128, 3, 2, 256], F, tag="Gog", name="Gog")
            FCp = fp.tile([128, 2, 2, 256], F, tag="FC")
            nc.vector.memset(FCp[:], 0.0)

            def fc_region(j, r0, r1, c0=0, c1=BF):
                return FCp[r0:r1, j // 2, j % 2, c0:c1]

            def mm(out, lhsT, rhs, start, stop):
                nc.tensor.matmul(out, lhsT, rhs, start=start, stop=stop,
                                 skip_group_check=True)

            def emit_x(s_, xa, gates=("i", "f", "o", "g")):
                # x matmuls (L0): 4 chunk-passes per gate accumulating [0:126]
                for gname in gates:
                    _, hf = _gate_region(gname, 0)
                    gtile = Gif if gname in ("i", "f") else Gog
                    for p in range(4):
                        ncp = XCH[p]
                        mm(gtile[0:126, 0, hf, 0:BF],
                           wap("x%s%d" % (gname, p), 0, 20 * ncp, 0, 126),
                           xa[0:20 * ncp, p, :], start=(p == 0), stop=False)

            def stream_chain(s_, k, l0, l1):
                """One stream's per-stage recurrence on cols 98k:98k+98."""
                c0, c1 = BS * k, BS * k + BS
                Hp = Hs[k][s_ % 2]        # h(s-1) inputs
                Hn = Hs[k][(s_ + 1) % 2]  # h(s) outputs

                def hmm(gset, gtile):
                    for gname in gset:
                        for l in range(l0, l1 + 1):
                            _, hf = _gate_region(gname, l)
                            out = gtile[0:126, l, hf, c0:c1]
                            x_open = (l == 0 and s_ < SEQ)
                            if l == 0:
                                mm(out, wap("h0%s" % gname, 0, 127, 0, 126),
                                   Hp[0:127, 0, :], start=not x_open, stop=True)
                            else:
                                nm = ("a1", "b1") if l == 1 else ("a2", "b2")
                                mm(out,
                                   wap("%s%s" % (nm[0], gname), 0, 127, 0, 126),
                                   Hp[0:127, l - 1, :], start=True, stop=False)
                                mm(out,
                                   wap("%s%s" % (nm[1], gname), 0, 126, 0, 126),
                                   Hp[0:126, l, :], start=False, stop=True)

                hmm(("i", "f"), Gif)
                hmm(("o", "g"), Gog)
                # sigmoid(i|f) across live layers: one fused op
                nc.scalar.activation(out=Sif[k][0:126, l0:l1 + 1, :, :],
                                     in_=Gif[0:126, l0:l1 + 1, 0:2, c0:c1],
                                     func=AF.Sigmoid)
                # z_f = sig_f*c right after sigmoid (no tanh_g dep)
                nc.vector.tensor_mul(out=Zt[k][0:126, l0:l1 + 1, 1:2, :],
                                     in0=Sif[k][0:126, l0:l1 + 1, 1:2, :],
                                     in1=Tt[k][0:126, l0:l1 + 1, 1:2, :])
                # tanh(g) -> T slot 0; sigmoid(o) -> So
                nc.scalar.activation(out=Tt[k][0:126, l0:l1 + 1, 0:1, :],
                                     in_=Gog[0:126, l0:l1 + 1, 1:2, c0:c1],
                                     func=AF.Tanh)
                nc.scalar.activation(out=So[k][0:126, l0:l1 + 1, :],
                                     in_=Gog[0:126, l0:l1 + 1, 0, c0:c1],
                                     func=AF.Sigmoid)
                # z_i = sig_i*tanh_g; c' = z_i + z_f; h = sig_o*tanh(c')
                nc.vector.tensor_mul(out=Zt[k][0:126, l0:l1 + 1, 0:1, :],
                                     in0=Sif[k][0:126, l0:l1 + 1, 0:1, :],
                                     in1=Tt[k][0:126, l0:l1 + 1, 0:1, :])
                nc.vector.tensor_add(out=Tt[k][0:126, l0:l1 + 1, 1:2, :],
                                     in0=Zt[k][0:126, l0:l1 + 1, 0:1, :],
                                     in1=Zt[k][0:126, l0:l1 + 1, 1:2, :])
                nc.scalar.activation(out=TC[k][0:126, l0:l1 + 1, :],
                                     in_=Tt[k][0:126, l0:l1 + 1, 1, :],
                                     func=AF.Tanh)
                nc.vector.tensor_mul(out=Hn[0:126, l0:l1 + 1, :],
                                     in0=So[k][0:126, l0:l1 + 1, :],
                                     in1=TC[k][0:126, l0:l1 + 1, :])

            xtiles = {}

            def emit_fc(t2):
                for k in range(2):
                    for j in range(4):
                        ncj = FCG[j]
                        mm(fc_region(j, 0, 20 * ncj, BS * k, BS * k + BS),
                           wap("fc%d_%d" % (t2, j), 0, 127, 0, 20 * ncj),
                           Hs[k][(t2 + 3) % 2][0:127, 2, :],
                           start=(t2 == 0), stop=(t2 == SEQ - 1))

            for s_ in range(SEQ + 2):
                l0, l1 = max(0, s_ - (SEQ - 1)), min(2, s_)
                # FC first in PE FIFO: it is always ready (double-buffered h)
                # and runs while the h-matmuls still wait on h(s-1)
                if 0 <= s_ - 4 < SEQ:
                    emit_fc(s_ - 4)
                if s_ == 0:
                    for t in (0, 1):
                        xa = xp.tile([120, 4, BF], H16, tag="xa",
                                     name="xa%d" % t)
                        nc.gpsimd.dma_start(out=xa[:], in_=xd[t])
                        xtiles[t] = xa
                    emit_x(0, xtiles[0])
                stream_chain(s_, 0, l0, l1)
                stream_chain(s_, 1, l0, l1)
                if s_ + 1 < SEQ:
                    emit_x(s_ + 1, xtiles[s_ + 1])
                if s_ + 2 < SEQ:
                    xa = xp.tile([120, 4, BF], H16, tag="xa",
                                 name="xa%d" % (s_ + 2))
                    nc.gpsimd.dma_start(out=xa[:], in_=xd[s_ + 2])
                    xtiles[s_ + 2] = xa
            # flush the last FC steps (t2 emitted at s_ = t2+4 > SEQ+1)
            for t2 in (SEQ - 2, SEQ - 1):
                emit_fc(t2)

            # ---- log_softmax tail (logits O(1); skip max subtraction)
            Lsb = sp.tile([128, 2, 2, BF], F, tag="Lsb")
            Esb = sp.tile([128, 2, 2, BF], H16, tag="Esb")
            nc.scalar.activation(out=Lsb[0:120, :, :, :],
                                 in_=FCp[0:120, 0:2, 0:2, 0:BF],
                                 func=AF.Identity)
            nc.scalar.activation(out=Esb[0:120, :, :, :],
                                 in_=FCp[0:120, 0:2, 0:2, 0:BF], func=AF.Exp)
            s_ps = gp.tile([21, BF], F, tag="Gif", name="Gsum")
            for j in range(4):
                ncj = FCG[j]
                mm(s_ps[0:21, :], wap("redK%d" % j, 0, 20 * ncj, 0, NB),
                   Esb[0:20 * ncj, j // 2, j % 2, :],
                   start=(j == 0), stop=(j == 3))
            lnz = sp.tile([21, BF], H16, tag="lnz")
            nc.scalar.activation(out=lnz[:], in_=s_ps[0:21, :], func=AF.Ln)
            bc = gp.tile([128, 2, 2, 256], F, tag="Gog", name="Gbc")
            for j in range(4):
                ncj = FCG[j]
                mm(bc[0:20 * ncj, j // 2, j % 2, 0:BF],
                   wap("redM%d" % j, 0, NB, 0, 20 * ncj),
                   lnz[0:21, :], start=True, stop=True)
            Osb = sp.tile([128, 2, 2, BF], F, tag="Osb")
            nc.vector.scalar_tensor_tensor(
                out=Osb[0:120, :, :, :], in0=bc[0:120, 0:2, 0:2, 0:BF],
                scalar=-1.0, in1=Lsb[0:120, :, :, :],
                op0=Alu.mult, op1=Alu.add)
            nc.sync.dma_start(out=od[:], in_=Osb[0:120, :, :, :])
    nc.compile()
    return nc


def _get_program(inputs):
    w_ih = [inputs["w_ih%d" % l] for l in range(3)]
    w_hh = [inputs["w_hh%d" % l] for l in range(3)]
    b_ih = [inputs["b_ih%d" % l] for l in range(3)]
    b_hh = [inputs["b_hh%d" % l] for l in range(3)]
    blob, col = _build_wblob(w_ih, w_hh, b_ih, b_hh,
                             inputs["fc_w"], inputs["fc_b"])
    if "nc1" not in _CACHE:
        _CACHE["nc1"] = _make_nc(blob.shape[1], col)
    return _CACHE["nc1"], blob


def kernel(**inputs):
    from concourse.bass_utils import run_bass_kernel_spmd

    nc, blob = _get_program(inputs)
    x = np.asarray(inputs["x"], dtype=np.float32)
    in_maps = []
    for c in range(NCORES):
        xc = x[c * BC:(c + 1) * BC, 0]  # (4096, 20, 34)
        in_maps.append({"xin": _prep_x(xc), "win": blob})
    res = run_bass_kernel_spmd(nc, in_maps, list(range(NCORES)),
                               trace=_CACHE.get("trace", False))
    _CACHE["last_res"] = res
    out = np.empty((B_TOTAL, CLS), dtype=np.float32)
    for c in range(NCORES):
        out[c * BC:(c + 1) * BC] = _unpack_out(res.results[c]["oout"])
    return out
